# revision 6
# baseline (speedup 1.0000x reference)
"""Autoformer-style EncoderLayer (series-decomp + single-head attention + FFN)
for Trainium2, data-parallel over batch across 8 NeuronCores.

Per core: one [L=2048, D=512] sequence.
  trend = AvgPool1d(x, k=25, pad=12, count_include_pad=True)
  s     = x - trend                        (banded matmul: S = B @ x, B = I - A)
  Q,K,V = s@wq+bq, s@wk+bk, s@wv+bv
  attn  = softmax(Q K^T / sqrt(D))         (computed transposed: scores^T[m,l])
  h     = LN1(s + attn@V@wo + bo)
  out   = LN2(h + relu(h@w1+bb1)@w2+bb2) + trend

All matmuls run in float32r (fp32 data at bf16 PE rate). Activations flow
"transposed" [d, l] for chained projections; scores are computed transposed so
softmax denominators come from ones-matmuls; attn@V yields AVT [d, l] whose Wo
projection lands back in natural [l, d] layout for the free-dim LayerNorms.
Natural<->transposed layout switches are f32r matmuls against an identity.
trend / seasonal / h are spilled to DRAM to keep SBUF under budget.
"""
import math
import numpy as np
from contextlib import ExitStack

import concourse.bass as bass
import concourse.mybir as mybir
import concourse.tile as tile
from concourse import bacc
from concourse.bass_utils import run_bass_kernel_spmd

P = 128
B_, L, D = 8, 2048, 512
KPOOL, PAD = 25, 12
EPS = 1e-5
SCALE = 1.0 / math.sqrt(D)
NLC = L // P          # 16 l-chunks of 128
NB = L // 512         # 4  l-blocks of 512
ND = D // P           # 4  d-chunks of 128

f32 = mybir.dt.float32
f32r = mybir.dt.float32r
AF = mybir.ActivationFunctionType
ALU = mybir.AluOpType

_CACHE = {}


def _band_blocks():
    i = np.arange(P)[:, None]
    j = np.arange(P)[None, :]
    a = (np.abs(i - j) <= PAD).astype(np.float32) / KPOOL
    bdiag = np.eye(P, dtype=np.float32) - a
    bup = -((i - j) >= (P - PAD)).astype(np.float32) / KPOOL   # rows chunk c-1, cols chunk c
    bdown = bup.T.copy()                                       # rows chunk c+1, cols chunk c
    return bdiag, bup, bdown


def _ln_small(nc, small, t_sum, t_ssq, t_eps):
    """Per-block LayerNorm stats on [P, 4]: returns (istd, -mean*istd)."""
    t_mean = small.tile([P, 4], f32, tag="lnm", name="t_mean")
    nc.vector.tensor_scalar_mul(t_mean[:], t_sum[:], 1.0 / D)
    t_ex2 = small.tile([P, 4], f32, tag="lne", name="t_ex2")
    nc.vector.tensor_scalar_mul(t_ex2[:], t_ssq[:], 1.0 / D)
    t_m2 = small.tile([P, 4], f32, tag="lnm2", name="t_m2")
    nc.vector.tensor_tensor(t_m2[:], t_mean[:], t_mean[:], ALU.mult)
    t_var = small.tile([P, 4], f32, tag="lnv", name="t_var")
    nc.vector.tensor_tensor(t_var[:], t_ex2[:], t_m2[:], ALU.subtract)
    t_sd = small.tile([P, 4], f32, tag="lnsd", name="t_sd")
    nc.scalar.activation(t_sd[:], t_var[:], AF.Sqrt, bias=t_eps[:])
    t_istd = small.tile([P, 4], f32, tag="lni", name="t_istd")
    nc.vector.reciprocal(t_istd[:], t_sd[:])
    t_mi = small.tile([P, 4], f32, tag="lnmi", name="t_mi")
    nc.vector.tensor_tensor(t_mi[:], t_mean[:], t_istd[:], ALU.mult)
    t_nmi = small.tile([P, 4], f32, tag="lnn", name="t_nmi")
    nc.vector.tensor_scalar_mul(t_nmi[:], t_mi[:], -1.0)
    return t_istd, t_nmi


def _build(apply_g1, apply_g2):
    nc = bacc.Bacc("TRN2", target_bir_lowering=False, debug=False)

    def din(name, shape):
        return nc.dram_tensor(name, list(shape), f32, kind="ExternalInput").ap()

    x = din("x", (L, D))
    ws = {n: din(n, (D, D)) for n in ["wq", "wk", "wv", "wo", "w1", "w2"]}
    bqk = {n: din(n, (P, ND)) for n in ["bq", "bk", "b1"]}
    rows = {n: din(n, (1, D)) for n in ["bv_row", "bo_row", "bb2_row"]}
    bdiag = din("bdiag", (P, P))
    bup = din("bup", (P, P))
    bdown = din("bdown", (P, P))
    iden = din("iden", (P, P))
    ones_col = din("ones_col", (P, 1))
    ones_row = din("ones_row", (1, P))
    ones2 = din("ones2", (1, 2))
    eps_col = din("eps_col", (P, 1))
    gb = {}
    if apply_g1:
        gb["g1b"] = din("g1b", (P, D))
        gb["be1b"] = din("be1b", (P, D))
    if apply_g2:
        gb["g2b"] = din("g2b", (P, D))
        gb["be2b"] = din("be2b", (P, D))

    out = nc.dram_tensor("out", [L, D], f32, kind="ExternalOutput").ap()
    out_c = out.rearrange("(l p) d -> l p d", p=P)

    with tile.TileContext(nc) as tc, ExitStack() as ctx:
        misc = ctx.enter_context(tc.tile_pool(name="misc", bufs=1))
        small = ctx.enter_context(tc.tile_pool(name="small", bufs=4))
        dram = ctx.enter_context(tc.tile_pool(name="dram", bufs=1, space="DRAM"))
        ps_mm = ctx.enter_context(tc.tile_pool(name="ps_mm", bufs=3, space="PSUM"))

        # ---- constants ----
        t_bd = misc.tile([P, P], f32r); nc.sync.dma_start(t_bd[:], bdiag.bitcast(f32r))
        t_bu = misc.tile([P, P], f32r); nc.sync.dma_start(t_bu[:], bup.bitcast(f32r))
        t_bn = misc.tile([P, P], f32r); nc.sync.dma_start(t_bn[:], bdown.bitcast(f32r))
        t_id = misc.tile([P, P], f32r); nc.sync.dma_start(t_id[:], iden.bitcast(f32r))
        t_oc = misc.tile([P, 1], f32r); nc.sync.dma_start(t_oc[:], ones_col.bitcast(f32r))
        t_or = misc.tile([1, P], f32r); nc.sync.dma_start(t_or[:], ones_row.bitcast(f32r))
        t_o2 = misc.tile([1, 2], f32r); nc.sync.dma_start(t_o2[:], ones2.bitcast(f32r))
        t_eps = misc.tile([P, 1], f32); nc.sync.dma_start(t_eps[:], eps_col[:])
        t_b = {}
        for n in ["bq", "bk", "b1"]:
            t_b[n] = misc.tile([P, ND], f32, name=f"t_{n}")
            nc.sync.dma_start(t_b[n][:], bqk[n][:])
        t_row = {}
        for n in ["bv_row", "bo_row", "bb2_row"]:
            t_row[n] = misc.tile([1, D], f32r, name=f"t_{n}")
            nc.sync.dma_start(t_row[n][:], rows[n].bitcast(f32r))
        t_gb = {}
        for n in gb:
            t_gb[n] = misc.tile([P, D], f32, name=f"t_{n}")
            nc.sync.dma_start(t_gb[n][:], gb[n][:])

        trend_sp = dram.tile([NLC, P, D], f32)
        s_sp = dram.tile([NLC, P, D], f32)
        h_sp = dram.tile([NLC, P, D], f32r)

        es_st, es_qkv = ExitStack(), ExitStack()
        try:
            # pool stack (LIFO): qkv > st > [sx phase1/2] / [wqkv phase3]
            qkv = es_qkv.enter_context(tc.tile_pool(name="qkv", bufs=1))
            t_qt = qkv.tile([P, ND, L], f32r, name="t_qt")
            t_kt = qkv.tile([P, ND, L], f32r, name="t_kt")
            t_v = qkv.tile([P, NLC, D], f32r, name="t_v")
            st_pool = es_st.enter_context(tc.tile_pool(name="stp", bufs=1))
            t_st = st_pool.tile([P, ND, L], f32r, name="t_st")

            # ---- phase 1+2: seasonal, trend, ST (x streamed in a window) ----
            x_cview = x.rearrange("(l p) d -> l p d", p=P)
            with tc.tile_pool(name="sxp", bufs=1) as sx_pool, \
                 tc.tile_pool(name="xwin", bufs=4) as xwin, \
                 tc.tile_pool(name="bstage", bufs=3) as bstage, \
                 tc.tile_pool(name="ps_t", bufs=2, space="PSUM") as ps_t:
                x_ch = {}

                def get_x(j):
                    if j not in x_ch:
                        t = xwin.tile([P, D], f32r, tag="xw", name=f"xw{j}")
                        nc.sync.dma_start(t[:], x_cview[j].bitcast(f32r))
                        x_ch[j] = t
                    return x_ch[j]

                t_s = sx_pool.tile([P, NLC, D], f32r, name="t_s")
                for lc in range(NLC):
                    pss = ps_mm.tile([P, D], f32, tag="mm", name="pss")
                    nbrs = [(lc - 1, t_bu), (lc, t_bd), (lc + 1, t_bn)]
                    nbrs = [(j, t) for j, t in nbrs if 0 <= j < NLC]
                    for i, (j, tb) in enumerate(nbrs):
                        nc.tensor.matmul(pss[:], tb[:], get_x(j)[:],
                                         start=(i == 0), stop=(i == len(nbrs) - 1))
                    nc.scalar.copy(t_s[:, lc, :], pss[:])
                    t_tr = bstage.tile([P, D], f32, tag="trst", name="t_tr")
                    nc.vector.tensor_tensor(t_tr[:], get_x(lc)[:].bitcast(f32),
                                            t_s[:, lc, :].bitcast(f32), ALU.subtract)
                    nc.sync.dma_start(trend_sp[lc], t_tr[:])
                    nc.sync.dma_start(s_sp[lc], t_s[:, lc, :].bitcast(f32))

                # ST = S^T via identity matmuls
                for lc in range(NLC):
                    for dc in range(ND):
                        pst = ps_t.tile([P, P], f32, tag="pt", name="pst")
                        nc.tensor.matmul(pst[:], t_s[:, lc, bass.ts(dc, P)], t_id[:],
                                         start=True, stop=True)
                        nc.scalar.copy(t_st[:, dc, bass.ts(lc, P)], pst[:])
            # s freed (kept in DRAM spill)

            # ---- phase 3: QKV projections + V ----
            with tc.tile_pool(name="wqkv", bufs=1) as wqkv:
                t_w = {}
                for n in ["wq", "wk", "wv"]:
                    t_w[n] = wqkv.tile([P, ND, D], f32r, name=f"t_w_{n}")
                    nc.sync.dma_start(
                        t_w[n][:], ws[n].rearrange("(k p) n -> p k n", p=P).bitcast(f32r))

                for tdst, wname, bname in [(t_qt, "wq", "bq"), (t_kt, "wk", "bk")]:
                    for dc in range(ND):
                        for lb in range(NB):
                            pq = ps_mm.tile([P, 512], f32, tag="mm", name="pq")
                            for k in range(ND):
                                nc.tensor.matmul(pq[:], t_w[wname][:, k, bass.ts(dc, P)],
                                                 t_st[:, k, bass.ts(lb, 512)],
                                                 start=(k == 0), stop=(k == ND - 1))
                            nc.scalar.activation(tdst[:, dc, bass.ts(lb, 512)], pq[:],
                                                 AF.Identity, bias=t_b[bname][:, dc:dc + 1])
                for mc in range(NLC):
                    pv = ps_mm.tile([P, D], f32, tag="mm", name="pv")
                    for k in range(ND):
                        nc.tensor.matmul(pv[:], t_st[:, k, bass.ts(mc, P)],
                                         t_w["wv"][:, k, :],
                                         start=(k == 0), stop=False)
                    nc.tensor.matmul(pv[:], t_or[:], t_row["bv_row"][:],
                                     start=False, stop=True)
                    nc.scalar.copy(t_v[:, mc, :], pv[:])
            es_st.close()  # ST freed

            # ---- phase 4: attention + LN1 ----
            with tc.tile_pool(name="upool", bufs=17) as upool, \
                 tc.tile_pool(name="wo_pool", bufs=1) as wo_pool, \
                 tc.tile_pool(name="astage", bufs=2) as astage, \
                 tc.tile_pool(name="avtp", bufs=2) as avtp, \
                 tc.tile_pool(name="ps_den", bufs=2, space="PSUM") as ps_den, \
                 tc.tile_pool(name="ps_rec", bufs=2, space="PSUM") as ps_rec:
                t_wo = wo_pool.tile([P, ND, D], f32r, name="t_wo")
                nc.sync.dma_start(
                    t_wo[:], ws["wo"].rearrange("(k p) n -> p k n", p=P).bitcast(f32r))

                for lb in range(NB):
                    u_tiles = []
                    for mc in range(NLC):
                        psc = ps_mm.tile([P, 512], f32, tag="mm", name="psc")
                        for k in range(ND):
                            nc.tensor.matmul(psc[:], t_kt[:, k, bass.ts(mc, P)],
                                             t_qt[:, k, bass.ts(lb, 512)],
                                             start=(k == 0), stop=(k == ND - 1))
                        ut = upool.tile([P, 512], f32r, tag="u", name=f"u_{lb}_{mc}")
                        nc.scalar.activation(ut[:], psc[:], AF.Exp, scale=SCALE)
                        u_tiles.append(ut)

                    pden = ps_den.tile([1, 512], f32, tag="den", name="pden")
                    for mc in range(NLC):
                        nc.tensor.matmul(pden[:], t_oc[:], u_tiles[mc][:],
                                         start=(mc == 0), stop=(mc == NLC - 1))
                    den_row = small.tile([1, 512], f32r, tag="denr", name="den_row")
                    nc.scalar.copy(den_row[:], pden[:])

                    prc = ps_rec.tile([P, 4, 2], f32, tag="rec", name="prc")
                    for c in range(4):
                        nc.tensor.matmul(prc[:, c, :], den_row[:, bass.ts(c, P)],
                                         t_o2[:], start=True, stop=True)
                    t_rec = small.tile([P, 4], f32, tag="recs", name="t_rec")
                    nc.vector.reciprocal(t_rec[:], prc[:, :, 0])

                    t_avt = avtp.tile([P, ND, 512], f32r, tag="avt", name="t_avt")
                    for dc in range(ND):
                        pav = ps_mm.tile([P, 512], f32, tag="mm", name="pav")
                        for mc in range(NLC):
                            nc.tensor.matmul(pav[:], t_v[:, mc, bass.ts(dc, P)],
                                             u_tiles[mc][:],
                                             start=(mc == 0), stop=(mc == NLC - 1))
                        nc.scalar.copy(t_avt[:, dc, :], pav[:])

                    t_sum1 = small.tile([P, 4], f32, tag="sum1", name="t_sum1")
                    t_ssq1 = small.tile([P, 4], f32, tag="ssq1", name="t_ssq1")
                    resids = []
                    for c in range(4):
                        lc = lb * 4 + c
                        pwo = ps_mm.tile([P, D], f32, tag="mm", name="pwo")
                        for k in range(ND):
                            nc.tensor.matmul(pwo[:], t_avt[:, k, bass.ts(c, P)],
                                             t_wo[:, k, :],
                                             start=(k == 0), stop=False)
                        nc.tensor.matmul(pwo[:], den_row[:, bass.ts(c, P)],
                                         t_row["bo_row"][:], start=False, stop=True)
                        t_srel = astage.tile([P, D], f32, tag="srel", name="t_srel")
                        nc.sync.dma_start(t_srel[:], s_sp[lc])
                        t_res = astage.tile([P, D], f32, tag="res1", bufs=5,
                                            name="t_res1")
                        nc.vector.scalar_tensor_tensor(
                            t_res[:], pwo[:], t_rec[:, c:c + 1], t_srel[:],
                            op0=ALU.mult, op1=ALU.add, accum_out=t_sum1[:, c:c + 1])
                        t_scr = astage.tile([P, D], f32, tag="sqscr", name="t_scr")
                        nc.scalar.activation(t_scr[:], t_res[:], AF.Square,
                                             accum_out=t_ssq1[:, c:c + 1])
                        resids.append(t_res)

                    t_istd, t_nmi = _ln_small(nc, small, t_sum1, t_ssq1, t_eps)
                    for c in range(4):
                        lc = lb * 4 + c
                        t_h = astage.tile([P, D], f32r, tag="hout", name="t_hh")
                        nc.scalar.activation(t_h[:], resids[c][:], AF.Identity,
                                             scale=t_istd[:, c:c + 1],
                                             bias=t_nmi[:, c:c + 1])
                        if apply_g1:
                            nc.vector.tensor_tensor(t_h[:].bitcast(f32),
                                                    t_h[:].bitcast(f32),
                                                    t_gb["g1b"][:], ALU.mult)
                            nc.vector.tensor_tensor(t_h[:].bitcast(f32),
                                                    t_h[:].bitcast(f32),
                                                    t_gb["be1b"][:], ALU.add)
                        nc.sync.dma_start(h_sp[lc], t_h[:])
            es_qkv.close()  # QT/KT/V freed

            # ---- phase 5: FFN + LN2 + trend ----
            with tc.tile_pool(name="ffn", bufs=1) as ffn, \
                 tc.tile_pool(name="fstage", bufs=2) as fstage, \
                 tc.tile_pool(name="ps_t2", bufs=2, space="PSUM") as ps_t2:
                t_h = ffn.tile([P, NLC, D], f32r, name="t_hf")
                for lc in range(NLC):
                    nc.sync.dma_start(t_h[:, lc, :], h_sp[lc])
                t_w1 = ffn.tile([P, ND, D], f32r, name="t_w1f")
                t_w2 = ffn.tile([P, ND, D], f32r, name="t_w2f")
                nc.sync.dma_start(
                    t_w1[:], ws["w1"].rearrange("(k p) n -> p k n", p=P).bitcast(f32r))
                nc.sync.dma_start(
                    t_w2[:], ws["w2"].rearrange("(k p) n -> p k n", p=P).bitcast(f32r))

                t_ht = ffn.tile([P, ND, L], f32r, name="t_htf")
                for lc in range(NLC):
                    for dc in range(ND):
                        pht = ps_t2.tile([P, P], f32, tag="pt2", name="pht")
                        nc.tensor.matmul(pht[:], t_h[:, lc, bass.ts(dc, P)], t_id[:],
                                         start=True, stop=True)
                        nc.scalar.copy(t_ht[:, dc, bass.ts(lc, P)], pht[:])

                t_rt = ffn.tile([P, ND, L], f32r, name="t_rtf")
                for dc in range(ND):
                    for lb in range(NB):
                        pf = ps_mm.tile([P, 512], f32, tag="mm", name="pf")
                        for k in range(ND):
                            nc.tensor.matmul(pf[:], t_w1[:, k, bass.ts(dc, P)],
                                             t_ht[:, k, bass.ts(lb, 512)],
                                             start=(k == 0), stop=(k == ND - 1))
                        nc.scalar.activation(t_rt[:, dc, bass.ts(lb, 512)], pf[:],
                                             AF.Relu, bias=t_b["b1"][:, dc:dc + 1])

                for lb in range(NB):
                    t_sum2 = small.tile([P, 4], f32, tag="sum2", name="t_sum2")
                    t_ssq2 = small.tile([P, 4], f32, tag="ssq2", name="t_ssq2")
                    resids = []
                    for c in range(4):
                        lc = lb * 4 + c
                        pf2 = ps_mm.tile([P, D], f32, tag="mm", name="pf2")
                        for k in range(ND):
                            nc.tensor.matmul(pf2[:], t_rt[:, k, bass.ts(lc, P)],
                                             t_w2[:, k, :],
                                             start=(k == 0), stop=False)
                        nc.tensor.matmul(pf2[:], t_or[:], t_row["bb2_row"][:],
                                         start=False, stop=True)
                        t_res = fstage.tile([P, D], f32, tag="res2", bufs=5,
                                            name="t_res2")
                        nc.vector.scalar_tensor_tensor(
                            t_res[:], pf2[:], 1.0, t_h[:, lc, :].bitcast(f32),
                            op0=ALU.mult, op1=ALU.add, accum_out=t_sum2[:, c:c + 1])
                        t_scr = fstage.tile([P, D], f32, tag="sqscr2", name="t_scr2")
                        nc.scalar.activation(t_scr[:], t_res[:], AF.Square,
                                             accum_out=t_ssq2[:, c:c + 1])
                        resids.append(t_res)

                    t_istd, t_nmi = _ln_small(nc, small, t_sum2, t_ssq2, t_eps)
                    for c in range(4):
                        lc = lb * 4 + c
                        t_h2 = fstage.tile([P, D], f32, tag="h2out", name="t_h2")
                        nc.scalar.activation(t_h2[:], resids[c][:], AF.Identity,
                                             scale=t_istd[:, c:c + 1],
                                             bias=t_nmi[:, c:c + 1])
                        if apply_g2:
                            nc.vector.tensor_tensor(t_h2[:], t_h2[:],
                                                    t_gb["g2b"][:], ALU.mult)
                            nc.vector.tensor_tensor(t_h2[:], t_h2[:],
                                                    t_gb["be2b"][:], ALU.add)
                        t_trel = fstage.tile([P, D], f32, tag="trel", name="t_trel")
                        nc.sync.dma_start(t_trel[:], trend_sp[lc])
                        t_out = fstage.tile([P, D], f32, tag="outst", name="t_out")
                        nc.vector.tensor_tensor(t_out[:], t_h2[:], t_trel[:], ALU.add)
                        nc.sync.dma_start(out_c[lc], t_out[:])
        finally:
            es_st.close(); es_qkv.close()

    nc.compile()
    return nc


def _consts(inp):
    bdiag, bup, bdown = _band_blocks()
    return {
        "wq": inp["wq"], "wk": inp["wk"], "wv": inp["wv"],
        "wo": inp["wo"], "w1": inp["w1"], "w2": inp["w2"],
        "bq": np.ascontiguousarray(inp["bq"].reshape(ND, P).T),
        "bk": np.ascontiguousarray(inp["bk"].reshape(ND, P).T),
        "b1": np.ascontiguousarray(inp["bb1"].reshape(ND, P).T),
        "bv_row": inp["bv"].reshape(1, D),
        "bo_row": inp["bo"].reshape(1, D),
        "bb2_row": inp["bb2"].reshape(1, D),
        "bdiag": bdiag, "bup": bup, "bdown": bdown,
        "iden": np.eye(P, dtype=np.float32),
        "ones_col": np.ones((P, 1), np.float32),
        "ones_row": np.ones((1, P), np.float32),
        "ones2": np.ones((1, 2), np.float32),
        "eps_col": np.full((P, 1), EPS, np.float32),
    }


def kernel(**inputs):
    inp = {k: np.ascontiguousarray(np.asarray(v, dtype=np.float32))
           for k, v in inputs.items()}
    x = inp["x"]                      # [8, 2048, 512]
    assert x.shape == (B_, L, D)

    apply_g1 = not (np.allclose(inp["g1"], 1.0) and np.allclose(inp["be1"], 0.0))
    apply_g2 = not (np.allclose(inp["g2"], 1.0) and np.allclose(inp["be2"], 0.0))

    key = (apply_g1, apply_g2)
    if key not in _CACHE:
        _CACHE[key] = _build(apply_g1, apply_g2)
    nc = _CACHE[key]

    consts = _consts(inp)
    if apply_g1:
        consts["g1b"] = np.tile(inp["g1"].reshape(1, D), (P, 1))
        consts["be1b"] = np.tile(inp["be1"].reshape(1, D), (P, 1))
    if apply_g2:
        consts["g2b"] = np.tile(inp["g2"].reshape(1, D), (P, 1))
        consts["be2b"] = np.tile(inp["be2"].reshape(1, D), (P, 1))
    consts = {k: np.ascontiguousarray(v, dtype=np.float32) for k, v in consts.items()}

    in_maps = [dict(consts, x=np.ascontiguousarray(x[i])) for i in range(B_)]
    res = run_bass_kernel_spmd(nc, in_maps, core_ids=list(range(B_)))
    return np.stack([res.results[i]["out"] for i in range(B_)], axis=0)


# revision 7
# speedup vs baseline: 1.0244x; 1.0244x over previous
"""Autoformer-style EncoderLayer (series-decomp + single-head attention + FFN)
for Trainium2, data-parallel over batch across 8 NeuronCores.

Per core: one [L=2048, D=512] sequence.
  trend = AvgPool1d(x, k=25, pad=12, count_include_pad=True)
  s     = x - trend                        (banded matmul: S = B @ x, B = I - A)
  Q,K,V = s@wq+bq, s@wk+bk, s@wv+bv
  attn  = softmax(Q K^T / sqrt(D))         (computed transposed: scores^T[m,l])
  h     = LN1(s + attn@V@wo + bo)
  out   = LN2(h + relu(h@w1+bb1)@w2+bb2) + trend

All matmuls run in float32r (fp32 data at bf16 PE rate). Activations flow
"transposed" [d, l] for chained projections; scores are computed transposed so
softmax denominators come from ones-matmuls; attn@V yields AVT [d, l] whose Wo
projection lands back in natural [l, d] layout for the free-dim LayerNorms.
Natural<->transposed layout switches are f32r matmuls against an identity.
trend / seasonal / h are spilled to DRAM to keep SBUF under budget.
"""
import math
import numpy as np
from contextlib import ExitStack

import concourse.bass as bass
import concourse.mybir as mybir
import concourse.tile as tile
from concourse import bacc
from concourse.bass_utils import run_bass_kernel_spmd

P = 128
B_, L, D = 8, 2048, 512
KPOOL, PAD = 25, 12
EPS = 1e-5
SCALE = 1.0 / math.sqrt(D)
NLC = L // P          # 16 l-chunks of 128
NB = L // 512         # 4  l-blocks of 512
ND = D // P           # 4  d-chunks of 128

f32 = mybir.dt.float32
f32r = mybir.dt.float32r
AF = mybir.ActivationFunctionType
ALU = mybir.AluOpType

_CACHE = {}


def _band_blocks():
    i = np.arange(P)[:, None]
    j = np.arange(P)[None, :]
    a = (np.abs(i - j) <= PAD).astype(np.float32) / KPOOL
    bdiag = np.eye(P, dtype=np.float32) - a
    bup = -((i - j) >= (P - PAD)).astype(np.float32) / KPOOL   # rows chunk c-1, cols chunk c
    bdown = bup.T.copy()                                       # rows chunk c+1, cols chunk c
    return bdiag, bup, bdown


def _ln_small(nc, small, t_sum, t_ssq, t_eps):
    """Per-block LayerNorm stats on [P, 4]: returns (istd, -mean*istd)."""
    t_mean = small.tile([P, 4], f32, tag="lnm", name="t_mean")
    nc.vector.tensor_scalar_mul(t_mean[:], t_sum[:], 1.0 / D)
    t_ex2 = small.tile([P, 4], f32, tag="lne", name="t_ex2")
    nc.vector.tensor_scalar_mul(t_ex2[:], t_ssq[:], 1.0 / D)
    t_m2 = small.tile([P, 4], f32, tag="lnm2", name="t_m2")
    nc.vector.tensor_tensor(t_m2[:], t_mean[:], t_mean[:], ALU.mult)
    t_var = small.tile([P, 4], f32, tag="lnv", name="t_var")
    nc.vector.tensor_tensor(t_var[:], t_ex2[:], t_m2[:], ALU.subtract)
    t_sd = small.tile([P, 4], f32, tag="lnsd", name="t_sd")
    nc.scalar.activation(t_sd[:], t_var[:], AF.Sqrt, bias=t_eps[:])
    t_istd = small.tile([P, 4], f32, tag="lni", name="t_istd")
    nc.vector.reciprocal(t_istd[:], t_sd[:])
    t_mi = small.tile([P, 4], f32, tag="lnmi", name="t_mi")
    nc.vector.tensor_tensor(t_mi[:], t_mean[:], t_istd[:], ALU.mult)
    t_nmi = small.tile([P, 4], f32, tag="lnn", name="t_nmi")
    nc.vector.tensor_scalar_mul(t_nmi[:], t_mi[:], -1.0)
    return t_istd, t_nmi


def _build(apply_g1, apply_g2):
    nc = bacc.Bacc("TRN2", target_bir_lowering=False, debug=False)

    def din(name, shape):
        return nc.dram_tensor(name, list(shape), f32, kind="ExternalInput").ap()

    x = din("x", (L, D))
    ws = {n: din(n, (D, D)) for n in ["wq", "wk", "wv", "wo", "w1", "w2"]}
    bqk = {n: din(n, (P, ND)) for n in ["bq", "bk", "b1"]}
    rows = {n: din(n, (1, D)) for n in ["bv_row", "bo_row", "bb2_row"]}
    bdiag = din("bdiag", (P, P))
    bup = din("bup", (P, P))
    bdown = din("bdown", (P, P))
    iden = din("iden", (P, P))
    ones_col = din("ones_col", (P, 1))
    ones_row = din("ones_row", (1, P))
    ones2 = din("ones2", (1, 2))
    eps_col = din("eps_col", (P, 1))
    gb = {}
    if apply_g1:
        gb["g1b"] = din("g1b", (P, D))
        gb["be1b"] = din("be1b", (P, D))
    if apply_g2:
        gb["g2b"] = din("g2b", (P, D))
        gb["be2b"] = din("be2b", (P, D))

    out = nc.dram_tensor("out", [L, D], f32, kind="ExternalOutput").ap()
    out_c = out.rearrange("(l p) d -> l p d", p=P)

    with tile.TileContext(nc) as tc, ExitStack() as ctx:
        misc = ctx.enter_context(tc.tile_pool(name="misc", bufs=1))
        small = ctx.enter_context(tc.tile_pool(name="small", bufs=4))
        dram = ctx.enter_context(tc.tile_pool(name="dram", bufs=1, space="DRAM"))
        ps_mm = ctx.enter_context(tc.tile_pool(name="ps_mm", bufs=4, space="PSUM"))

        # ---- constants ----
        t_bd = misc.tile([P, P], f32r); nc.sync.dma_start(t_bd[:], bdiag.bitcast(f32r))
        t_bu = misc.tile([P, P], f32r); nc.sync.dma_start(t_bu[:], bup.bitcast(f32r))
        t_bn = misc.tile([P, P], f32r); nc.sync.dma_start(t_bn[:], bdown.bitcast(f32r))
        t_id = misc.tile([P, P], f32r); nc.sync.dma_start(t_id[:], iden.bitcast(f32r))
        t_oc = misc.tile([P, 1], f32r); nc.sync.dma_start(t_oc[:], ones_col.bitcast(f32r))
        t_or = misc.tile([1, P], f32r); nc.sync.dma_start(t_or[:], ones_row.bitcast(f32r))
        t_o2 = misc.tile([1, 2], f32r); nc.sync.dma_start(t_o2[:], ones2.bitcast(f32r))
        t_eps = misc.tile([P, 1], f32); nc.sync.dma_start(t_eps[:], eps_col[:])
        t_b = {}
        for n in ["bq", "bk", "b1"]:
            t_b[n] = misc.tile([P, ND], f32, name=f"t_{n}")
            nc.sync.dma_start(t_b[n][:], bqk[n][:])
        t_row = {}
        for n in ["bv_row", "bo_row", "bb2_row"]:
            t_row[n] = misc.tile([1, D], f32r, name=f"t_{n}")
            nc.sync.dma_start(t_row[n][:], rows[n].bitcast(f32r))
        t_gb = {}
        for n in gb:
            t_gb[n] = misc.tile([P, D], f32, name=f"t_{n}")
            nc.sync.dma_start(t_gb[n][:], gb[n][:])

        trend_sp = dram.tile([NLC, P, D], f32)
        s_sp = dram.tile([NLC, P, D], f32)
        h_sp = dram.tile([NLC, P, D], f32r)

        es_qkv = ExitStack()
        try:
            qkv = es_qkv.enter_context(tc.tile_pool(name="qkv", bufs=1))
            t_qt = qkv.tile([P, ND, L], f32r, name="t_qt")
            t_kt = qkv.tile([P, ND, L], f32r, name="t_kt")
            t_v = qkv.tile([P, NLC, D], f32r, name="t_v")

            # ---- phases 1-3 fused, streamed per l-block:
            # banded seasonal -> trend -> S^T block -> QT/KT/V block
            x_cview = x.rearrange("(l p) d -> p l d", p=P)
            with tc.tile_pool(name="wqkv", bufs=1) as wqkv, \
                 tc.tile_pool(name="xwin", bufs=8) as xwin, \
                 tc.tile_pool(name="sstr", bufs=6) as sstr, \
                 tc.tile_pool(name="stbp", bufs=2) as stbp, \
                 tc.tile_pool(name="bstage", bufs=3) as bstage, \
                 tc.tile_pool(name="ps_t", bufs=2, space="PSUM") as ps_t:
                t_w = {}
                for n in ["wq", "wk", "wv"]:
                    t_w[n] = wqkv.tile([P, ND, D], f32r, name=f"t_w_{n}")
                    nc.sync.dma_start(
                        t_w[n][:], ws[n].rearrange("(k p) n -> p k n", p=P).bitcast(f32r))

                x_ch = {}

                def get_x(j):
                    if j not in x_ch:
                        t = xwin.tile([P, D], f32r, tag="xw", name=f"xw{j}")
                        nc.sync.dma_start(t[:], x_cview[:, j, :].bitcast(f32r))
                        x_ch[j] = t
                    return x_ch[j]

                for lb in range(NB):
                    # (a) banded S + trend for this block's 4 chunks
                    s_chunks = []
                    for c in range(4):
                        lc = lb * 4 + c
                        pss = ps_mm.tile([P, D], f32, tag="mm", name="pss")
                        nbrs = [(lc - 1, t_bu), (lc, t_bd), (lc + 1, t_bn)]
                        nbrs = [(j, t) for j, t in nbrs if 0 <= j < NLC]
                        for i, (j, tb) in enumerate(nbrs):
                            nc.tensor.matmul(pss[:], tb[:], get_x(j)[:],
                                             start=(i == 0), stop=(i == len(nbrs) - 1))
                        t_sc = sstr.tile([P, D], f32r, tag="s", name=f"s_{lc}")
                        nc.scalar.copy(t_sc[:], pss[:])
                        s_chunks.append(t_sc)
                        t_tr = bstage.tile([P, D], f32, tag="trst", name="t_tr")
                        nc.vector.tensor_tensor(t_tr[:], get_x(lc)[:].bitcast(f32),
                                                t_sc[:].bitcast(f32), ALU.subtract)
                        nc.sync.dma_start(trend_sp[lc], t_tr[:])
                        nc.sync.dma_start(s_sp[lc], t_sc[:].bitcast(f32))

                    # (b) S^T block [d, l-block] via identity matmuls
                    stb = stbp.tile([P, ND, 512], f32r, tag="stb", name="stb")
                    for c in range(4):
                        for dc in range(ND):
                            pst = ps_t.tile([P, P], f32, tag="pt", name="pst")
                            nc.tensor.matmul(pst[:], s_chunks[c][:, bass.ts(dc, P)],
                                             t_id[:], start=True, stop=True)
                            nc.scalar.copy(stb[:, dc, bass.ts(c, P)], pst[:])

                    # (c) QT/KT for this l-block
                    for tdst, wname, bname in [(t_qt, "wq", "bq"), (t_kt, "wk", "bk")]:
                        for dc in range(ND):
                            pq = ps_mm.tile([P, 512], f32, tag="mm", name="pq")
                            for k in range(ND):
                                nc.tensor.matmul(pq[:], t_w[wname][:, k, bass.ts(dc, P)],
                                                 stb[:, k, :],
                                                 start=(k == 0), stop=(k == ND - 1))
                            nc.scalar.activation(tdst[:, dc, bass.ts(lb, 512)], pq[:],
                                                 AF.Identity, bias=t_b[bname][:, dc:dc + 1])
                    # (d) V for this block's 4 m-chunks
                    for c in range(4):
                        mc = lb * 4 + c
                        pv = ps_mm.tile([P, D], f32, tag="mm", name="pv")
                        for k in range(ND):
                            nc.tensor.matmul(pv[:], stb[:, k, bass.ts(c, P)],
                                             t_w["wv"][:, k, :],
                                             start=(k == 0), stop=False)
                        nc.tensor.matmul(pv[:], t_or[:], t_row["bv_row"][:],
                                         start=False, stop=True)
                        nc.scalar.copy(t_v[:, mc, :], pv[:])

            # ---- phase 4: attention + LN1 ----
            with tc.tile_pool(name="upool", bufs=20) as upool, \
                 tc.tile_pool(name="wo_pool", bufs=1) as wo_pool, \
                 tc.tile_pool(name="astage", bufs=2) as astage, \
                 tc.tile_pool(name="avtp", bufs=2) as avtp, \
                 tc.tile_pool(name="ps_den", bufs=2, space="PSUM") as ps_den, \
                 tc.tile_pool(name="ps_rec", bufs=2, space="PSUM") as ps_rec:
                t_wo = wo_pool.tile([P, ND, D], f32r, name="t_wo")
                nc.sync.dma_start(
                    t_wo[:], ws["wo"].rearrange("(k p) n -> p k n", p=P).bitcast(f32r))

                for lb in range(NB):
                    u_tiles = []
                    for mc in range(NLC):
                        psc = ps_mm.tile([P, 512], f32, tag="mm", name="psc")
                        for k in range(ND):
                            nc.tensor.matmul(psc[:], t_kt[:, k, bass.ts(mc, P)],
                                             t_qt[:, k, bass.ts(lb, 512)],
                                             start=(k == 0), stop=(k == ND - 1))
                        ut = upool.tile([P, 512], f32r, tag="u", name=f"u_{lb}_{mc}")
                        nc.scalar.activation(ut[:], psc[:], AF.Exp, scale=SCALE)
                        u_tiles.append(ut)

                    pden = ps_den.tile([1, 512], f32, tag="den", name="pden")
                    for mc in range(NLC):
                        nc.tensor.matmul(pden[:], t_oc[:], u_tiles[mc][:],
                                         start=(mc == 0), stop=(mc == NLC - 1))
                    den_row = small.tile([1, 512], f32r, tag="denr", name="den_row")
                    nc.scalar.copy(den_row[:], pden[:])

                    prc = ps_rec.tile([P, 4, 2], f32, tag="rec", name="prc")
                    for c in range(4):
                        nc.tensor.matmul(prc[:, c, :], den_row[:, bass.ts(c, P)],
                                         t_o2[:], start=True, stop=True)
                    t_rec = small.tile([P, 4], f32, tag="recs", name="t_rec")
                    nc.vector.reciprocal(t_rec[:], prc[:, :, 0])

                    t_avt = avtp.tile([P, ND, 512], f32r, tag="avt", name="t_avt")
                    for dc in range(ND):
                        pav = ps_mm.tile([P, 512], f32, tag="mm", name="pav")
                        for mc in range(NLC):
                            nc.tensor.matmul(pav[:], t_v[:, mc, bass.ts(dc, P)],
                                             u_tiles[mc][:],
                                             start=(mc == 0), stop=(mc == NLC - 1))
                        nc.scalar.copy(t_avt[:, dc, :], pav[:])

                    t_sum1 = small.tile([P, 4], f32, tag="sum1", name="t_sum1")
                    t_ssq1 = small.tile([P, 4], f32, tag="ssq1", name="t_ssq1")
                    resids = []
                    for c in range(4):
                        lc = lb * 4 + c
                        pwo = ps_mm.tile([P, D], f32, tag="mm", name="pwo")
                        for k in range(ND):
                            nc.tensor.matmul(pwo[:], t_avt[:, k, bass.ts(c, P)],
                                             t_wo[:, k, :],
                                             start=(k == 0), stop=False)
                        nc.tensor.matmul(pwo[:], den_row[:, bass.ts(c, P)],
                                         t_row["bo_row"][:], start=False, stop=True)
                        t_srel = astage.tile([P, D], f32, tag="srel", name="t_srel")
                        nc.sync.dma_start(t_srel[:], s_sp[lc])
                        t_res = astage.tile([P, D], f32, tag="res1", bufs=5,
                                            name="t_res1")
                        nc.vector.scalar_tensor_tensor(
                            t_res[:], pwo[:], t_rec[:, c:c + 1], t_srel[:],
                            op0=ALU.mult, op1=ALU.add, accum_out=t_sum1[:, c:c + 1])
                        t_scr = astage.tile([P, D], f32, tag="sqscr", name="t_scr")
                        nc.scalar.activation(t_scr[:], t_res[:], AF.Square,
                                             accum_out=t_ssq1[:, c:c + 1])
                        resids.append(t_res)

                    t_istd, t_nmi = _ln_small(nc, small, t_sum1, t_ssq1, t_eps)
                    for c in range(4):
                        lc = lb * 4 + c
                        t_h = astage.tile([P, D], f32r, tag="hout", name="t_hh")
                        nc.scalar.activation(t_h[:], resids[c][:], AF.Identity,
                                             scale=t_istd[:, c:c + 1],
                                             bias=t_nmi[:, c:c + 1])
                        if apply_g1:
                            nc.vector.tensor_tensor(t_h[:].bitcast(f32),
                                                    t_h[:].bitcast(f32),
                                                    t_gb["g1b"][:], ALU.mult)
                            nc.vector.tensor_tensor(t_h[:].bitcast(f32),
                                                    t_h[:].bitcast(f32),
                                                    t_gb["be1b"][:], ALU.add)
                        nc.sync.dma_start(h_sp[lc], t_h[:])
            es_qkv.close()  # QT/KT/V freed

            # ---- phase 5: FFN + LN2 + trend ----
            with tc.tile_pool(name="ffn", bufs=1) as ffn, \
                 tc.tile_pool(name="fstage", bufs=2) as fstage, \
                 tc.tile_pool(name="ps_t2", bufs=2, space="PSUM") as ps_t2:
                t_h = ffn.tile([P, NLC, D], f32r, name="t_hf")
                for lc in range(NLC):
                    nc.sync.dma_start(t_h[:, lc, :], h_sp[lc])
                t_w1 = ffn.tile([P, ND, D], f32r, name="t_w1f")
                t_w2 = ffn.tile([P, ND, D], f32r, name="t_w2f")
                nc.sync.dma_start(
                    t_w1[:], ws["w1"].rearrange("(k p) n -> p k n", p=P).bitcast(f32r))
                nc.sync.dma_start(
                    t_w2[:], ws["w2"].rearrange("(k p) n -> p k n", p=P).bitcast(f32r))

                t_ht = ffn.tile([P, ND, L], f32r, name="t_htf")
                for lc in range(NLC):
                    for dc in range(ND):
                        pht = ps_t2.tile([P, P], f32, tag="pt2", name="pht")
                        nc.tensor.matmul(pht[:], t_h[:, lc, bass.ts(dc, P)], t_id[:],
                                         start=True, stop=True)
                        nc.scalar.copy(t_ht[:, dc, bass.ts(lc, P)], pht[:])

                t_rt = ffn.tile([P, ND, L], f32r, name="t_rtf")
                for dc in range(ND):
                    for lb in range(NB):
                        pf = ps_mm.tile([P, 512], f32, tag="mm", name="pf")
                        for k in range(ND):
                            nc.tensor.matmul(pf[:], t_w1[:, k, bass.ts(dc, P)],
                                             t_ht[:, k, bass.ts(lb, 512)],
                                             start=(k == 0), stop=(k == ND - 1))
                        nc.scalar.activation(t_rt[:, dc, bass.ts(lb, 512)], pf[:],
                                             AF.Relu, bias=t_b["b1"][:, dc:dc + 1])

                for lb in range(NB):
                    t_sum2 = small.tile([P, 4], f32, tag="sum2", name="t_sum2")
                    t_ssq2 = small.tile([P, 4], f32, tag="ssq2", name="t_ssq2")
                    resids = []
                    for c in range(4):
                        lc = lb * 4 + c
                        pf2 = ps_mm.tile([P, D], f32, tag="mm", name="pf2")
                        for k in range(ND):
                            nc.tensor.matmul(pf2[:], t_rt[:, k, bass.ts(lc, P)],
                                             t_w2[:, k, :],
                                             start=(k == 0), stop=False)
                        nc.tensor.matmul(pf2[:], t_or[:], t_row["bb2_row"][:],
                                         start=False, stop=True)
                        t_res = fstage.tile([P, D], f32, tag="res2", bufs=5,
                                            name="t_res2")
                        nc.vector.scalar_tensor_tensor(
                            t_res[:], pf2[:], 1.0, t_h[:, lc, :].bitcast(f32),
                            op0=ALU.mult, op1=ALU.add, accum_out=t_sum2[:, c:c + 1])
                        t_scr = fstage.tile([P, D], f32, tag="sqscr2", name="t_scr2")
                        nc.scalar.activation(t_scr[:], t_res[:], AF.Square,
                                             accum_out=t_ssq2[:, c:c + 1])
                        resids.append(t_res)

                    t_istd, t_nmi = _ln_small(nc, small, t_sum2, t_ssq2, t_eps)
                    for c in range(4):
                        lc = lb * 4 + c
                        t_h2 = fstage.tile([P, D], f32, tag="h2out", name="t_h2")
                        nc.scalar.activation(t_h2[:], resids[c][:], AF.Identity,
                                             scale=t_istd[:, c:c + 1],
                                             bias=t_nmi[:, c:c + 1])
                        if apply_g2:
                            nc.vector.tensor_tensor(t_h2[:], t_h2[:],
                                                    t_gb["g2b"][:], ALU.mult)
                            nc.vector.tensor_tensor(t_h2[:], t_h2[:],
                                                    t_gb["be2b"][:], ALU.add)
                        t_trel = fstage.tile([P, D], f32, tag="trel", name="t_trel")
                        nc.sync.dma_start(t_trel[:], trend_sp[lc])
                        t_out = fstage.tile([P, D], f32, tag="outst", name="t_out")
                        nc.vector.tensor_tensor(t_out[:], t_h2[:], t_trel[:], ALU.add)
                        nc.sync.dma_start(out_c[lc], t_out[:])
        finally:
            es_qkv.close()

    nc.compile()
    return nc


def _consts(inp):
    bdiag, bup, bdown = _band_blocks()
    return {
        "wq": inp["wq"], "wk": inp["wk"], "wv": inp["wv"],
        "wo": inp["wo"], "w1": inp["w1"], "w2": inp["w2"],
        "bq": np.ascontiguousarray(inp["bq"].reshape(ND, P).T),
        "bk": np.ascontiguousarray(inp["bk"].reshape(ND, P).T),
        "b1": np.ascontiguousarray(inp["bb1"].reshape(ND, P).T),
        "bv_row": inp["bv"].reshape(1, D),
        "bo_row": inp["bo"].reshape(1, D),
        "bb2_row": inp["bb2"].reshape(1, D),
        "bdiag": bdiag, "bup": bup, "bdown": bdown,
        "iden": np.eye(P, dtype=np.float32),
        "ones_col": np.ones((P, 1), np.float32),
        "ones_row": np.ones((1, P), np.float32),
        "ones2": np.ones((1, 2), np.float32),
        "eps_col": np.full((P, 1), EPS, np.float32),
    }


def kernel(**inputs):
    inp = {k: np.ascontiguousarray(np.asarray(v, dtype=np.float32))
           for k, v in inputs.items()}
    x = inp["x"]                      # [8, 2048, 512]
    assert x.shape == (B_, L, D)

    apply_g1 = not (np.allclose(inp["g1"], 1.0) and np.allclose(inp["be1"], 0.0))
    apply_g2 = not (np.allclose(inp["g2"], 1.0) and np.allclose(inp["be2"], 0.0))

    key = (apply_g1, apply_g2)
    if key not in _CACHE:
        _CACHE[key] = _build(apply_g1, apply_g2)
    nc = _CACHE[key]

    consts = _consts(inp)
    if apply_g1:
        consts["g1b"] = np.tile(inp["g1"].reshape(1, D), (P, 1))
        consts["be1b"] = np.tile(inp["be1"].reshape(1, D), (P, 1))
    if apply_g2:
        consts["g2b"] = np.tile(inp["g2"].reshape(1, D), (P, 1))
        consts["be2b"] = np.tile(inp["be2"].reshape(1, D), (P, 1))
    consts = {k: np.ascontiguousarray(v, dtype=np.float32) for k, v in consts.items()}

    in_maps = [dict(consts, x=np.ascontiguousarray(x[i])) for i in range(B_)]
    res = run_bass_kernel_spmd(nc, in_maps, core_ids=list(range(B_)))
    return np.stack([res.results[i]["out"] for i in range(B_)], axis=0)


# revision 26
# speedup vs baseline: 1.1489x; 1.1216x over previous
"""Autoformer-style EncoderLayer (series-decomp + single-head attention + FFN)
for Trainium2, data-parallel over batch across 8 NeuronCores.

Per core: one [L=2048, D=512] sequence.
  trend = AvgPool1d(x, k=25, pad=12, count_include_pad=True)
  s     = x - trend                        (banded matmul: S = B @ x, B = I - A)
  Q,K,V = s@wq+bq, s@wk+bk, s@wv+bv
  attn  = softmax(Q K^T / sqrt(D))         (computed transposed: scores^T[m,l])
  h     = LN1(s + attn@V@wo + bo)
  out   = LN2(h + relu(h@w1+bb1)@w2+bb2) + trend

All matmuls run in float32r (fp32 data at bf16 PE rate). Activations flow
"transposed" [d, l] for chained projections; scores are computed transposed so
softmax denominators come from ones-matmuls; attn@V yields AVT [d, l] whose Wo
projection lands back in natural [l, d] layout for the free-dim LayerNorms.
Natural<->transposed layout switches are f32r matmuls against an identity.
trend / seasonal / h are spilled to DRAM to keep SBUF under budget.
"""
import math
import numpy as np
from contextlib import ExitStack

import concourse.bass as bass
import concourse.mybir as mybir
import concourse.tile as tile
from concourse import bacc
from concourse.bass_utils import run_bass_kernel_spmd

P = 128
B_, L, D = 8, 2048, 512
KPOOL, PAD = 25, 12
EPS = 1e-5
SCALE = 1.0 / math.sqrt(D)
NLC = L // P          # 16 l-chunks of 128
NB = L // 512         # 4  l-blocks of 512
ND = D // P           # 4  d-chunks of 128

f32 = mybir.dt.float32
f32r = mybir.dt.float32r
AF = mybir.ActivationFunctionType
ALU = mybir.AluOpType

_CACHE = {}


def _band_blocks():
    i = np.arange(P)[:, None]
    j = np.arange(P)[None, :]
    a = (np.abs(i - j) <= PAD).astype(np.float32) / KPOOL
    bdiag = np.eye(P, dtype=np.float32) - a
    bup = -((i - j) >= (P - PAD)).astype(np.float32) / KPOOL   # rows chunk c-1, cols chunk c
    bdown = bup.T.copy()                                       # rows chunk c+1, cols chunk c
    return bdiag, bup, bdown


def _ln_block(nc, small, t_sum, t_ssq, t_eps):
    """Per-block LayerNorm stats on [P, 4]: returns (istd, nmi, negmean)."""
    t_mean = small.tile([P, 4], f32, tag="lbm", name="tb_mean")
    nc.vector.tensor_scalar_mul(t_mean[:], t_sum[:], 1.0 / D)
    t_m2 = small.tile([P, 4], f32, tag="lbm2", name="tb_m2")
    nc.vector.tensor_tensor(t_m2[:], t_mean[:], t_mean[:], ALU.mult)
    t_var = small.tile([P, 4], f32, tag="lbv", name="tb_var")
    nc.vector.scalar_tensor_tensor(t_var[:], t_ssq[:], 1.0 / D, t_m2[:],
                                   op0=ALU.mult, op1=ALU.subtract)
    t_sd = small.tile([P, 4], f32, tag="lbsd", name="tb_sd")
    nc.scalar.activation(t_sd[:], t_var[:], AF.Sqrt, bias=t_eps[:])
    t_istd = small.tile([P, 4], f32, tag="lbi", name="tb_istd")
    nc.vector.reciprocal(t_istd[:], t_sd[:])
    t_nmi = small.tile([P, 4], f32, tag="lbn", name="tb_nmi")
    nc.vector.scalar_tensor_tensor(t_nmi[:], t_mean[:], -1.0, t_istd[:],
                                   op0=ALU.mult, op1=ALU.mult)
    t_negm = small.tile([P, 4], f32, tag="lbng", name="tb_negm")
    nc.vector.tensor_scalar_mul(t_negm[:], t_mean[:], -1.0)
    return t_istd, t_nmi, t_negm


def _ln_chunk(nc, small, t_sum, t_ssq, t_eps):
    """Per-chunk LayerNorm stats on [P, 1]: returns (istd, -mean*istd)."""
    t_mean = small.tile([P, 1], f32, tag="lnm", name="t_mean")
    nc.vector.tensor_scalar_mul(t_mean[:], t_sum[:], 1.0 / D)
    t_var = small.tile([P, 1], f32, tag="lnv", name="t_var")
    # var = ssq/D - mean^2  ->  (ssq/D) + (-mean*mean) via two ops
    t_m2 = small.tile([P, 1], f32, tag="lnm2", name="t_m2")
    nc.vector.tensor_tensor(t_m2[:], t_mean[:], t_mean[:], ALU.mult)
    nc.vector.scalar_tensor_tensor(t_var[:], t_ssq[:], 1.0 / D, t_m2[:],
                                   op0=ALU.mult, op1=ALU.subtract)
    t_sd = small.tile([P, 1], f32, tag="lnsd", name="t_sd")
    nc.scalar.activation(t_sd[:], t_var[:], AF.Sqrt, bias=t_eps[:])
    t_istd = small.tile([P, 1], f32, tag="lni", name="t_istd")
    nc.vector.reciprocal(t_istd[:], t_sd[:])
    t_nmi = small.tile([P, 1], f32, tag="lnn", name="t_nmi")
    nc.vector.scalar_tensor_tensor(t_nmi[:], t_mean[:], -1.0, t_istd[:],
                                   op0=ALU.mult, op1=ALU.mult)
    t_negm = small.tile([P, 1], f32, tag="lnng", name="t_negm")
    nc.vector.tensor_scalar_mul(t_negm[:], t_mean[:], -1.0)
    return t_istd, t_nmi, t_negm


def _build(apply_g1, apply_g2):
    nc = bacc.Bacc("TRN2", target_bir_lowering=False, debug=False)

    def din(name, shape):
        return nc.dram_tensor(name, list(shape), f32, kind="ExternalInput").ap()

    x = din("x", (L, D))
    ws = {n: din(n, (D, D)) for n in ["wq", "wk", "wv", "wo", "w1", "w2"]}
    cpack = din("cpack", (P, 526))
    rpack = din("rpack", (1, 1666))
    gb = {}
    if apply_g1:
        gb["g1b"] = din("g1b", (P, D))
        gb["be1b"] = din("be1b", (P, D))
    if apply_g2:
        gb["g2b"] = din("g2b", (P, D))
        gb["be2b"] = din("be2b", (P, D))

    out = nc.dram_tensor("out", [L, D], f32, kind="ExternalOutput").ap()
    out_c = out.rearrange("(l p) d -> l p d", p=P)

    with tile.TileContext(nc) as tc, ExitStack() as ctx:
        misc = ctx.enter_context(tc.tile_pool(name="misc", bufs=1))
        small = ctx.enter_context(tc.tile_pool(name="small", bufs=4))
        dram = ctx.enter_context(tc.tile_pool(name="dram", bufs=1, space="DRAM"))
        ps_mm = ctx.enter_context(tc.tile_pool(name="ps_mm", bufs=4, space="PSUM"))

        # ---- constants (two packed DMAs) ----
        t_cp = misc.tile([P, 526], f32r, name="t_cp")
        nc.sync.dma_start(t_cp[:], cpack.bitcast(f32r))
        t_rp = misc.tile([1, 1666], f32r, name="t_rp")
        nc.sync.dma_start(t_rp[:], rpack.bitcast(f32r))
        t_bd = t_cp[:, 0:128]
        t_bu = t_cp[:, 128:256]
        t_bn = t_cp[:, 256:384]
        t_id = t_cp[:, 384:512]
        t_oc = t_cp[:, 512:513]
        t_eps = t_cp[:, 513:514].bitcast(f32)
        t_row = {"bv_row": t_rp[:, 0:512], "bo_row": t_rp[:, 512:1024],
                 "bb2_row": t_rp[:, 1024:1536]}
        t_or = t_rp[:, 1536:1664]
        t_o2 = t_rp[:, 1664:1666]
        t_b = {"bq": t_cp[:, 514:518].bitcast(f32),
               "bk": t_cp[:, 518:522].bitcast(f32),
               "b1": t_cp[:, 522:526].bitcast(f32)}
        t_sum1a = misc.tile([P, NLC], f32, name="t_sum1a")
        t_ssq1a = misc.tile([P, NLC], f32, name="t_ssq1a")
        t_gb = {}
        for n in gb:
            t_gb[n] = misc.tile([P, D], f32, name=f"t_{n}")
            nc.sync.dma_start(t_gb[n][:], gb[n][:])

        trend_sp = dram.tile([NB, P, 4, D], f32)
        s_sp = dram.tile([NB, P, 4, D], f32)
        h_sp = dram.tile([NB, P, 4, D], f32)

        es_qkv = ExitStack()
        try:
            qkv = es_qkv.enter_context(tc.tile_pool(name="qkv", bufs=1))
            t_qt = qkv.tile([P, ND, L], f32r, name="t_qt")
            t_kt = qkv.tile([P, ND, L], f32r, name="t_kt")
            t_v = qkv.tile([P, NLC, D], f32r, name="t_v")

            # ---- phases 1-3 fused, streamed per l-block:
            # banded seasonal -> trend -> S^T block -> QT/KT/V block
            x_cview = x.rearrange("(l p) d -> p l d", p=P)
            with tc.tile_pool(name="wqkv", bufs=1) as wqkv, \
                 tc.tile_pool(name="xwin", bufs=6) as xwin, \
                 tc.tile_pool(name="sslab", bufs=2) as sslab, \
                 tc.tile_pool(name="trslab", bufs=2) as trslab, \
                 tc.tile_pool(name="stbp", bufs=2) as stbp, \
                 tc.tile_pool(name="ps_t", bufs=2, space="PSUM") as ps_t:
                x_ch = {}

                def get_x(j):
                    if j not in x_ch:
                        t = xwin.tile([P, D], f32r, tag="xw", name=f"xw{j}")
                        nc.sync.dma_start(t[:], x_cview[:, j, :].bitcast(f32r))
                        x_ch[j] = t
                    return x_ch[j]

                for j in range(5):      # prefetch ahead of the weight DMAs
                    get_x(j)
                t_w = {}
                for n in ["wq", "wk", "wv"]:
                    t_w[n] = wqkv.tile([P, ND, D], f32r, name=f"t_w_{n}")
                    nc.sync.dma_start(
                        t_w[n][:], ws[n].rearrange("(k p) n -> p k n", p=P).bitcast(f32r))

                for lb in range(NB):
                    # (a) banded S + trend into per-block slabs
                    s_slab = sslab.tile([P, 4, D], f32, tag="ss", name="s_slab")
                    tr_slab = trslab.tile([P, 4, D], f32, tag="tr", name="tr_slab")
                    for c in range(4):
                        lc = lb * 4 + c
                        pss = ps_mm.tile([P, D], f32, tag="mm", name="pss")
                        nbrs = [(lc - 1, t_bu), (lc, t_bd), (lc + 1, t_bn)]
                        nbrs = [(j, t) for j, t in nbrs if 0 <= j < NLC]
                        for i, (j, tb) in enumerate(nbrs):
                            nc.tensor.matmul(pss[:], tb[:], get_x(j)[:],
                                             start=(i == 0), stop=(i == len(nbrs) - 1))
                        nc.vector.tensor_copy(s_slab[:, c, :], pss[:])
                        nc.vector.tensor_tensor(tr_slab[:, c, :],
                                                get_x(lc)[:].bitcast(f32),
                                                s_slab[:, c, :],
                                                ALU.subtract)
                    nc.gpsimd.dma_start(s_sp[lb], s_slab[:])
                    nc.gpsimd.dma_start(trend_sp[lb], tr_slab[:])

                    # (b) S^T block [d, l-block] via identity matmuls
                    stb = stbp.tile([P, ND, 512], f32r, tag="stb", name="stb")
                    for c in range(4):
                        for dc in range(ND):
                            pst = ps_t.tile([P, P], f32, tag="pt", name="pst")
                            nc.tensor.matmul(pst[:], s_slab[:, c, bass.ts(dc, P)],
                                             t_id.bitcast(f32), start=True, stop=True)
                            nc.scalar.copy(stb[:, dc, bass.ts(c, P)], pst[:])

                    # (c) QT/KT for this l-block
                    for tdst, wname, bname in [(t_qt, "wq", "bq"), (t_kt, "wk", "bk")]:
                        for dc in range(ND):
                            pq = ps_mm.tile([P, 512], f32, tag="mm", name="pq")
                            for k in range(ND):
                                nc.tensor.matmul(pq[:], t_w[wname][:, k, bass.ts(dc, P)],
                                                 stb[:, k, :],
                                                 start=(k == 0), stop=(k == ND - 1))
                            nc.scalar.activation(tdst[:, dc, bass.ts(lb, 512)], pq[:],
                                                 AF.Identity, bias=t_b[bname][:, dc:dc + 1])
                    # (d) V for this block's 4 m-chunks
                    for c in range(4):
                        mc = lb * 4 + c
                        pv = ps_mm.tile([P, D], f32, tag="mm", name="pv")
                        for k in range(ND):
                            nc.tensor.matmul(pv[:], stb[:, k, bass.ts(c, P)],
                                             t_w["wv"][:, k, :],
                                             start=(k == 0), stop=False)
                        nc.tensor.matmul(pv[:], t_or[:], t_row["bv_row"][:],
                                         start=False, stop=True)
                        nc.scalar.copy(t_v[:, mc, :], pv[:])

            # ---- phase 4: attention + LN1 ----
            with tc.tile_pool(name="upool", bufs=18) as upool, \
                 tc.tile_pool(name="wo_pool", bufs=1) as wo_pool, \
                 tc.tile_pool(name="astage", bufs=2) as astage, \
                 tc.tile_pool(name="avtp", bufs=1) as avtp, \
                 tc.tile_pool(name="ps_den", bufs=2, space="PSUM") as ps_den, \
                 tc.tile_pool(name="ps_rec", bufs=2, space="PSUM") as ps_rec:
                t_wo = wo_pool.tile([P, ND, D], f32r, name="t_wo")
                nc.sync.dma_start(
                    t_wo[:], ws["wo"].rearrange("(k p) n -> p k n", p=P).bitcast(f32r))

                for lb in range(NB):
                    u_tiles = []
                    srel_slab = astage.tile([P, 4, D], f32, tag="srel", bufs=2,
                                            name="srel_slab")
                    nc.sync.dma_start(srel_slab[:], s_sp[lb])
                    rs_slab = astage.tile([P, 4, D], f32, tag="rs", bufs=2,
                                          name="rs_slab")
                    for mc in range(NLC):
                        psc = ps_mm.tile([P, 512], f32, tag="mm", name="psc")
                        for k in range(ND):
                            nc.tensor.matmul(psc[:], t_kt[:, k, bass.ts(mc, P)],
                                             t_qt[:, k, bass.ts(lb, 512)],
                                             start=(k == 0), stop=(k == ND - 1))
                        ut = upool.tile([P, 512], f32r, tag="u", name=f"u_{lb}_{mc}")
                        nc.scalar.activation(ut[:], psc[:], AF.Exp, scale=SCALE)
                        u_tiles.append(ut)

                    pden = ps_den.tile([1, 512], f32, tag="den", name="pden")
                    for mc in range(NLC):
                        nc.tensor.matmul(pden[:], t_oc[:], u_tiles[mc][:],
                                         start=(mc == 0), stop=(mc == NLC - 1))
                    den_row = small.tile([1, 512], f32r, tag="denr", name="den_row")
                    nc.scalar.copy(den_row[:], pden[:])

                    prc = ps_rec.tile([P, 4, 2], f32, tag="rec", name="prc")
                    for c in range(4):
                        nc.tensor.matmul(prc[:, c, :], den_row[:, bass.ts(c, P)],
                                         t_o2[:], start=True, stop=True)
                    t_rec = small.tile([P, 4], f32, tag="recs", name="t_rec")
                    nc.vector.reciprocal(t_rec[:], prc[:, :, 0])

                    t_avt = avtp.tile([P, ND, 512], f32r, tag="avt", name="t_avt")
                    for dc in range(ND):
                        pav = ps_mm.tile([P, 512], f32, tag="mm", name="pav")
                        for mc in range(NLC):
                            nc.tensor.matmul(pav[:], t_v[:, mc, bass.ts(dc, P)],
                                             u_tiles[mc][:],
                                             start=(mc == 0), stop=(mc == NLC - 1))
                        nc.scalar.copy(t_avt[:, dc, :], pav[:])

                    for c in range(4):
                        lc = lb * 4 + c
                        pwo = ps_mm.tile([P, D], f32, tag="mm", name="pwo")
                        for k in range(ND):
                            nc.tensor.matmul(pwo[:], t_avt[:, k, bass.ts(c, P)],
                                             t_wo[:, k, :],
                                             start=(k == 0), stop=False)
                        nc.tensor.matmul(pwo[:], den_row[:, bass.ts(c, P)],
                                         t_row["bo_row"][:], start=False, stop=True)
                        nc.vector.scalar_tensor_tensor(
                            rs_slab[:, c, :], pwo[:], t_rec[:, c:c + 1],
                            srel_slab[:, c, :],
                            op0=ALU.mult, op1=ALU.add,
                            accum_out=t_sum1a[:, lc:lc + 1])
                        t_scr = astage.tile([P, D], f32, tag="sqscr", name="t_scr")
                        nc.vector.scalar_tensor_tensor(
                            t_scr[:], rs_slab[:, c, :], 1.0, rs_slab[:, c, :],
                            op0=ALU.mult, op1=ALU.mult,
                            accum_out=t_ssq1a[:, lc:lc + 1])
                    nc.gpsimd.dma_start(h_sp[lb], rs_slab[:])
            es_qkv.close()  # QT/KT/V freed

            # ---- phase 5: FFN + LN2 + trend ----
            with tc.tile_pool(name="ffnw", bufs=1) as ffnw, \
                 tc.tile_pool(name="ffn", bufs=1) as ffn, \
                 tc.tile_pool(name="fstage", bufs=2) as fstage, \
                 tc.tile_pool(name="ps_t2", bufs=2, space="PSUM") as ps_t2:
                t_w1 = ffnw.tile([P, ND, D], f32r, name="t_w1f")
                t_w2 = ffnw.tile([P, ND, D], f32r, name="t_w2f")
                nc.sync.dma_start(
                    t_w1[:], ws["w1"].rearrange("(k p) n -> p k n", p=P).bitcast(f32r))
                nc.sync.dma_start(
                    t_w2[:], ws["w2"].rearrange("(k p) n -> p k n", p=P).bitcast(f32r))
                t_h = ffnw.tile([P, NLC, D], f32, name="t_hf")
                for lb in range(NB):
                  rrel_slab = fstage.tile([P, 4, D], f32, tag="rrel", bufs=2,
                                          name="rrel_slab")
                  nc.sync.dma_start(rrel_slab[:], h_sp[lb])
                  t_istd4, t_nmi4, t_negm4 = _ln_block(
                      nc, small, t_sum1a[:, lb * 4:lb * 4 + 4],
                      t_ssq1a[:, lb * 4:lb * 4 + 4], t_eps)
                  for c in range(4):
                    lc = lb * 4 + c
                    nc.vector.tensor_scalar(t_h[:, lc, :], rrel_slab[:, c, :],
                                            t_negm4[:, c:c + 1], t_istd4[:, c:c + 1],
                                            op0=ALU.add, op1=ALU.mult)
                    if apply_g1:
                        nc.vector.tensor_tensor(t_h[:, lc, :], t_h[:, lc, :],
                                                t_gb["g1b"][:], ALU.mult)
                        nc.vector.tensor_tensor(t_h[:, lc, :], t_h[:, lc, :],
                                                t_gb["be1b"][:], ALU.add)

                t_ht = ffn.tile([P, ND, L], f32r, name="t_htf")
                for lc in range(NLC):
                    for dc in range(ND):
                        pht = ps_t2.tile([P, P], f32, tag="pt2", name="pht")
                        nc.tensor.matmul(pht[:], t_h[:, lc, bass.ts(dc, P)],
                                         t_id.bitcast(f32), start=True, stop=True)
                        nc.scalar.copy(t_ht[:, dc, bass.ts(lc, P)], pht[:])

                t_rt = ffn.tile([P, ND, L], f32r, name="t_rtf")
                for lb in range(NB):
                    # ff1 for this l-block
                    for dc in range(ND):
                        pf = ps_mm.tile([P, 512], f32, tag="mm", name="pf")
                        for k in range(ND):
                            nc.tensor.matmul(pf[:], t_w1[:, k, bass.ts(dc, P)],
                                             t_ht[:, k, bass.ts(lb, 512)],
                                             start=(k == 0), stop=(k == ND - 1))
                        nc.scalar.activation(t_rt[:, dc, bass.ts(lb, 512)], pf[:],
                                             AF.Relu, bias=t_b["b1"][:, dc:dc + 1])
                    trel_slab = fstage.tile([P, 4, D], f32, tag="trel", bufs=2,
                                            name="trel_slab")
                    nc.sync.dma_start(trel_slab[:], trend_sp[lb])
                    t_sum2b = small.tile([P, 4], f32, tag="sum2b", name="t_sum2b")
                    t_ssq2b = small.tile([P, 4], f32, tag="ssq2b", name="t_ssq2b")
                    res_list = []
                    for c in range(4):
                        lc = lb * 4 + c
                        pf2 = ps_mm.tile([P, D], f32, tag="mm", name="pf2")
                        for k in range(ND):
                            nc.tensor.matmul(pf2[:], t_rt[:, k, bass.ts(lc, P)],
                                             t_w2[:, k, :],
                                             start=(k == 0), stop=False)
                        nc.tensor.matmul(pf2[:], t_or[:], t_row["bb2_row"][:],
                                         start=False, stop=True)
                        t_res = fstage.tile([P, D], f32, tag="res2", bufs=5,
                                            name="t_res2")
                        nc.vector.scalar_tensor_tensor(
                            t_res[:], pf2[:], 1.0, t_h[:, lc, :],
                            op0=ALU.mult, op1=ALU.add,
                            accum_out=t_sum2b[:, c:c + 1])
                        t_scr = fstage.tile([P, D], f32, tag="sqscr2", name="t_scr2")
                        nc.scalar.activation(t_scr[:], t_res[:], AF.Square,
                                             accum_out=t_ssq2b[:, c:c + 1])
                        res_list.append(t_res)
                    t_istd4, t_nmi4, t_negm4 = _ln_block(
                        nc, small, t_sum2b, t_ssq2b, t_eps)
                    for c in range(4):
                        lc = lb * 4 + c
                        t_h2 = fstage.tile([P, D], f32, tag="h2out", name="t_h2")
                        if c % 2 == 1:
                            nc.scalar.activation(t_h2[:], res_list[c][:], AF.Identity,
                                                 scale=t_istd4[:, c:c + 1],
                                                 bias=t_nmi4[:, c:c + 1])
                        else:
                            nc.vector.tensor_scalar(t_h2[:], res_list[c][:],
                                                    t_negm4[:, c:c + 1],
                                                    t_istd4[:, c:c + 1],
                                                    op0=ALU.add, op1=ALU.mult)
                        if apply_g2:
                            nc.vector.tensor_tensor(t_h2[:], t_h2[:],
                                                    t_gb["g2b"][:], ALU.mult)
                            nc.vector.tensor_tensor(t_h2[:], t_h2[:],
                                                    t_gb["be2b"][:], ALU.add)
                        t_out = fstage.tile([P, D], f32, tag="outst", name="t_out")
                        eng = nc.gpsimd if c % 2 == 0 else nc.vector
                        eng.tensor_tensor(t_out[:], t_h2[:], trel_slab[:, c, :],
                                          ALU.add)
                        nc.sync.dma_start(out_c[lc], t_out[:])
        finally:
            es_qkv.close()

    nc.compile()
    return nc


def _consts(inp):
    bdiag, bup, bdown = _band_blocks()
    cpack = np.zeros((P, 526), np.float32)
    cpack[:, 0:128] = bdiag
    cpack[:, 128:256] = bup
    cpack[:, 256:384] = bdown
    cpack[:, 384:512] = np.eye(P, dtype=np.float32)
    cpack[:, 512] = 1.0                      # ones_col
    cpack[:, 513] = EPS
    cpack[:, 514:518] = inp["bq"].reshape(ND, P).T
    cpack[:, 518:522] = inp["bk"].reshape(ND, P).T
    cpack[:, 522:526] = inp["bb1"].reshape(ND, P).T
    rpack = np.zeros((1, 1666), np.float32)
    rpack[0, 0:512] = inp["bv"]
    rpack[0, 512:1024] = inp["bo"]
    rpack[0, 1024:1536] = inp["bb2"]
    rpack[0, 1536:1664] = 1.0               # ones_row
    rpack[0, 1664:1666] = 1.0               # ones2
    return {
        "wq": inp["wq"], "wk": inp["wk"], "wv": inp["wv"],
        "wo": inp["wo"], "w1": inp["w1"], "w2": inp["w2"],
        "cpack": cpack, "rpack": rpack,
    }


def _prepare(inputs):
    inp = {k: np.ascontiguousarray(np.asarray(v, dtype=np.float32))
           for k, v in inputs.items()}
    x = inp["x"]                      # [8, 2048, 512]
    assert x.shape == (B_, L, D)

    apply_g1 = not (np.allclose(inp["g1"], 1.0) and np.allclose(inp["be1"], 0.0))
    apply_g2 = not (np.allclose(inp["g2"], 1.0) and np.allclose(inp["be2"], 0.0))

    key = (apply_g1, apply_g2)
    if key not in _CACHE:
        _CACHE[key] = _build(apply_g1, apply_g2)
    nc = _CACHE[key]

    consts = _consts(inp)
    if apply_g1:
        consts["g1b"] = np.tile(inp["g1"].reshape(1, D), (P, 1))
        consts["be1b"] = np.tile(inp["be1"].reshape(1, D), (P, 1))
    if apply_g2:
        consts["g2b"] = np.tile(inp["g2"].reshape(1, D), (P, 1))
        consts["be2b"] = np.tile(inp["be2"].reshape(1, D), (P, 1))
    consts = {k: np.ascontiguousarray(v, dtype=np.float32) for k, v in consts.items()}
    in_maps = [dict(consts, x=np.ascontiguousarray(x[i])) for i in range(B_)]
    return nc, in_maps


def kernel(**inputs):
    nc, in_maps = _prepare(inputs)
    res = run_bass_kernel_spmd(nc, in_maps, core_ids=list(range(B_)))
    return np.stack([res.results[i]["out"] for i in range(B_)], axis=0)


# revision 27
# speedup vs baseline: 1.1518x; 1.0025x over previous
"""Autoformer-style EncoderLayer (series-decomp + single-head attention + FFN)
for Trainium2, data-parallel over batch across 8 NeuronCores.

Per core: one [L=2048, D=512] sequence.
  trend = AvgPool1d(x, k=25, pad=12, count_include_pad=True)
  s     = x - trend                        (banded matmul: S = B @ x, B = I - A)
  Q,K,V = s@wq+bq, s@wk+bk, s@wv+bv
  attn  = softmax(Q K^T / sqrt(D))         (computed transposed: scores^T[m,l])
  h     = LN1(s + attn@V@wo + bo)
  out   = LN2(h + relu(h@w1+bb1)@w2+bb2) + trend

All matmuls run in float32r (fp32 data at bf16 PE rate). Activations flow
"transposed" [d, l] for chained projections; scores are computed transposed so
softmax denominators come from ones-matmuls; attn@V yields AVT [d, l] whose Wo
projection lands back in natural [l, d] layout for the free-dim LayerNorms.
Natural<->transposed layout switches are f32r matmuls against an identity.
trend / seasonal / h are spilled to DRAM to keep SBUF under budget.
"""
import math
import numpy as np
from contextlib import ExitStack

import concourse.bass as bass
import concourse.mybir as mybir
import concourse.tile as tile
from concourse import bacc
from concourse.bass_utils import run_bass_kernel_spmd

P = 128
B_, L, D = 8, 2048, 512
KPOOL, PAD = 25, 12
EPS = 1e-5
SCALE = 1.0 / math.sqrt(D)
NLC = L // P          # 16 l-chunks of 128
NB = L // 512         # 4  l-blocks of 512
ND = D // P           # 4  d-chunks of 128

f32 = mybir.dt.float32
f32r = mybir.dt.float32r
AF = mybir.ActivationFunctionType
ALU = mybir.AluOpType

_CACHE = {}


def _band_blocks():
    i = np.arange(P)[:, None]
    j = np.arange(P)[None, :]
    a = (np.abs(i - j) <= PAD).astype(np.float32) / KPOOL
    bdiag = np.eye(P, dtype=np.float32) - a
    bup = -((i - j) >= (P - PAD)).astype(np.float32) / KPOOL   # rows chunk c-1, cols chunk c
    bdown = bup.T.copy()                                       # rows chunk c+1, cols chunk c
    return bdiag, bup, bdown


def _ln_block(nc, small, t_sum, t_ssq, t_eps):
    """Per-block LayerNorm stats on [P, 4]: returns (istd, nmi, negmean)."""
    t_mean = small.tile([P, 4], f32, tag="lbm", name="tb_mean")
    nc.vector.tensor_scalar_mul(t_mean[:], t_sum[:], 1.0 / D)
    t_m2 = small.tile([P, 4], f32, tag="lbm2", name="tb_m2")
    nc.vector.tensor_tensor(t_m2[:], t_mean[:], t_mean[:], ALU.mult)
    t_var = small.tile([P, 4], f32, tag="lbv", name="tb_var")
    nc.vector.scalar_tensor_tensor(t_var[:], t_ssq[:], 1.0 / D, t_m2[:],
                                   op0=ALU.mult, op1=ALU.subtract)
    t_sd = small.tile([P, 4], f32, tag="lbsd", name="tb_sd")
    nc.scalar.activation(t_sd[:], t_var[:], AF.Sqrt, bias=t_eps[:])
    t_istd = small.tile([P, 4], f32, tag="lbi", name="tb_istd")
    nc.vector.reciprocal(t_istd[:], t_sd[:])
    t_nmi = small.tile([P, 4], f32, tag="lbn", name="tb_nmi")
    nc.vector.scalar_tensor_tensor(t_nmi[:], t_mean[:], -1.0, t_istd[:],
                                   op0=ALU.mult, op1=ALU.mult)
    t_negm = small.tile([P, 4], f32, tag="lbng", name="tb_negm")
    nc.vector.tensor_scalar_mul(t_negm[:], t_mean[:], -1.0)
    return t_istd, t_nmi, t_negm


def _ln_chunk(nc, small, t_sum, t_ssq, t_eps):
    """Per-chunk LayerNorm stats on [P, 1]: returns (istd, -mean*istd)."""
    t_mean = small.tile([P, 1], f32, tag="lnm", name="t_mean")
    nc.vector.tensor_scalar_mul(t_mean[:], t_sum[:], 1.0 / D)
    t_var = small.tile([P, 1], f32, tag="lnv", name="t_var")
    # var = ssq/D - mean^2  ->  (ssq/D) + (-mean*mean) via two ops
    t_m2 = small.tile([P, 1], f32, tag="lnm2", name="t_m2")
    nc.vector.tensor_tensor(t_m2[:], t_mean[:], t_mean[:], ALU.mult)
    nc.vector.scalar_tensor_tensor(t_var[:], t_ssq[:], 1.0 / D, t_m2[:],
                                   op0=ALU.mult, op1=ALU.subtract)
    t_sd = small.tile([P, 1], f32, tag="lnsd", name="t_sd")
    nc.scalar.activation(t_sd[:], t_var[:], AF.Sqrt, bias=t_eps[:])
    t_istd = small.tile([P, 1], f32, tag="lni", name="t_istd")
    nc.vector.reciprocal(t_istd[:], t_sd[:])
    t_nmi = small.tile([P, 1], f32, tag="lnn", name="t_nmi")
    nc.vector.scalar_tensor_tensor(t_nmi[:], t_mean[:], -1.0, t_istd[:],
                                   op0=ALU.mult, op1=ALU.mult)
    t_negm = small.tile([P, 1], f32, tag="lnng", name="t_negm")
    nc.vector.tensor_scalar_mul(t_negm[:], t_mean[:], -1.0)
    return t_istd, t_nmi, t_negm


def _build(apply_g1, apply_g2):
    nc = bacc.Bacc("TRN2", target_bir_lowering=False, debug=False)

    def din(name, shape):
        return nc.dram_tensor(name, list(shape), f32, kind="ExternalInput").ap()

    x = din("x", (L, D))
    ws = {n: din(n, (D, D)) for n in ["wq", "wk", "wv", "wo", "w1", "w2"]}
    cpack = din("cpack", (P, 526))
    rpack = din("rpack", (1, 1666))
    gb = {}
    if apply_g1:
        gb["g1b"] = din("g1b", (P, D))
        gb["be1b"] = din("be1b", (P, D))
    if apply_g2:
        gb["g2b"] = din("g2b", (P, D))
        gb["be2b"] = din("be2b", (P, D))

    out = nc.dram_tensor("out", [L, D], f32, kind="ExternalOutput").ap()
    out_c = out.rearrange("(l p) d -> l p d", p=P)

    with tile.TileContext(nc) as tc, ExitStack() as ctx:
        misc = ctx.enter_context(tc.tile_pool(name="misc", bufs=1))
        small = ctx.enter_context(tc.tile_pool(name="small", bufs=4))
        dram = ctx.enter_context(tc.tile_pool(name="dram", bufs=1, space="DRAM"))
        ps_mm = ctx.enter_context(tc.tile_pool(name="ps_mm", bufs=4, space="PSUM"))

        # ---- constants (two packed DMAs) ----
        t_cp = misc.tile([P, 526], f32r, name="t_cp")
        nc.sync.dma_start(t_cp[:], cpack.bitcast(f32r))
        t_rp = misc.tile([1, 1666], f32r, name="t_rp")
        nc.sync.dma_start(t_rp[:], rpack.bitcast(f32r))
        t_bd = t_cp[:, 0:128]
        t_bu = t_cp[:, 128:256]
        t_bn = t_cp[:, 256:384]
        t_id = t_cp[:, 384:512]
        t_oc = t_cp[:, 512:513]
        t_eps = t_cp[:, 513:514].bitcast(f32)
        t_row = {"bv_row": t_rp[:, 0:512], "bo_row": t_rp[:, 512:1024],
                 "bb2_row": t_rp[:, 1024:1536]}
        t_or = t_rp[:, 1536:1664]
        t_o2 = t_rp[:, 1664:1666]
        t_b = {"bq": t_cp[:, 514:518].bitcast(f32),
               "bk": t_cp[:, 518:522].bitcast(f32),
               "b1": t_cp[:, 522:526].bitcast(f32)}
        t_sum1a = misc.tile([P, NLC], f32, name="t_sum1a")
        t_ssq1a = misc.tile([P, NLC], f32, name="t_ssq1a")
        t_gb = {}
        for n in gb:
            t_gb[n] = misc.tile([P, D], f32, name=f"t_{n}")
            nc.sync.dma_start(t_gb[n][:], gb[n][:])

        trend_sp = dram.tile([NB, P, 4, D], f32)
        s_sp = dram.tile([NB, P, 4, D], f32)
        h_sp = dram.tile([NB, P, 4, D], f32)

        es_qkv = ExitStack()
        try:
            qkv = es_qkv.enter_context(tc.tile_pool(name="qkv", bufs=1))
            t_qt = qkv.tile([P, ND, L], f32r, name="t_qt")
            t_kt = qkv.tile([P, ND, L], f32r, name="t_kt")
            t_v = qkv.tile([P, NLC, D], f32r, name="t_v")

            # ---- phases 1-3 fused, streamed per l-block:
            # banded seasonal -> trend -> S^T block -> QT/KT/V block
            x_cview = x.rearrange("(l p) d -> p l d", p=P)
            with tc.tile_pool(name="wqkv", bufs=1) as wqkv, \
                 tc.tile_pool(name="xwin", bufs=6) as xwin, \
                 tc.tile_pool(name="sslab", bufs=2) as sslab, \
                 tc.tile_pool(name="trslab", bufs=2) as trslab, \
                 tc.tile_pool(name="stbp", bufs=2) as stbp, \
                 tc.tile_pool(name="ps_t", bufs=2, space="PSUM") as ps_t:
                x_ch = {}

                def get_x(j):
                    if j not in x_ch:
                        t = xwin.tile([P, D], f32r, tag="xw", name=f"xw{j}")
                        nc.sync.dma_start(t[:], x_cview[:, j, :].bitcast(f32r))
                        x_ch[j] = t
                    return x_ch[j]

                for j in range(5):      # prefetch ahead of the weight DMAs
                    get_x(j)
                t_w = {}
                for n in ["wq", "wk", "wv"]:
                    t_w[n] = wqkv.tile([P, ND, D], f32r, name=f"t_w_{n}")
                    nc.sync.dma_start(
                        t_w[n][:], ws[n].rearrange("(k p) n -> p k n", p=P).bitcast(f32r))

                for lb in range(NB):
                    # (a) banded S + trend into per-block slabs
                    s_slab = sslab.tile([P, 4, D], f32, tag="ss", name="s_slab")
                    tr_slab = trslab.tile([P, 4, D], f32, tag="tr", name="tr_slab")
                    for c in range(4):
                        lc = lb * 4 + c
                        pss = ps_mm.tile([P, D], f32, tag="mm", name="pss")
                        nbrs = [(lc - 1, t_bu), (lc, t_bd), (lc + 1, t_bn)]
                        nbrs = [(j, t) for j, t in nbrs if 0 <= j < NLC]
                        for i, (j, tb) in enumerate(nbrs):
                            nc.tensor.matmul(pss[:], tb[:], get_x(j)[:],
                                             start=(i == 0), stop=(i == len(nbrs) - 1))
                        nc.vector.tensor_copy(s_slab[:, c, :], pss[:])
                        nc.vector.tensor_tensor(tr_slab[:, c, :],
                                                get_x(lc)[:].bitcast(f32),
                                                s_slab[:, c, :],
                                                ALU.subtract)
                    nc.gpsimd.dma_start(s_sp[lb], s_slab[:])
                    nc.gpsimd.dma_start(trend_sp[lb], tr_slab[:])

                    # (b) S^T block [d, l-block] via identity matmuls
                    stb = stbp.tile([P, ND, 512], f32r, tag="stb", name="stb")
                    for c in range(4):
                        for dc in range(ND):
                            pst = ps_t.tile([P, P], f32, tag="pt", name="pst")
                            nc.tensor.matmul(pst[:], s_slab[:, c, bass.ts(dc, P)],
                                             t_id.bitcast(f32), start=True, stop=True)
                            nc.scalar.copy(stb[:, dc, bass.ts(c, P)], pst[:])

                    # (c) QT/KT for this l-block
                    for tdst, wname, bname in [(t_qt, "wq", "bq"), (t_kt, "wk", "bk")]:
                        for dc in range(ND):
                            pq = ps_mm.tile([P, 512], f32, tag="mm", name="pq")
                            for k in range(ND):
                                nc.tensor.matmul(pq[:], t_w[wname][:, k, bass.ts(dc, P)],
                                                 stb[:, k, :],
                                                 start=(k == 0), stop=(k == ND - 1))
                            nc.scalar.activation(tdst[:, dc, bass.ts(lb, 512)], pq[:],
                                                 AF.Identity, bias=t_b[bname][:, dc:dc + 1])
                    # (d) V for this block's 4 m-chunks
                    for c in range(4):
                        mc = lb * 4 + c
                        pv = ps_mm.tile([P, D], f32, tag="mm", name="pv")
                        for k in range(ND):
                            nc.tensor.matmul(pv[:], stb[:, k, bass.ts(c, P)],
                                             t_w["wv"][:, k, :],
                                             start=(k == 0), stop=False)
                        nc.tensor.matmul(pv[:], t_or[:], t_row["bv_row"][:],
                                         start=False, stop=True)
                        nc.scalar.copy(t_v[:, mc, :], pv[:])

            # ---- phase 4: attention + LN1 ----
            with tc.tile_pool(name="upool", bufs=18) as upool, \
                 tc.tile_pool(name="wo_pool", bufs=1) as wo_pool, \
                 tc.tile_pool(name="astage", bufs=2) as astage, \
                 tc.tile_pool(name="avtp", bufs=1) as avtp, \
                 tc.tile_pool(name="ps_den", bufs=2, space="PSUM") as ps_den, \
                 tc.tile_pool(name="ps_rec", bufs=2, space="PSUM") as ps_rec:
                t_wo = wo_pool.tile([P, ND, D], f32r, name="t_wo")
                nc.sync.dma_start(
                    t_wo[:], ws["wo"].rearrange("(k p) n -> p k n", p=P).bitcast(f32r))

                for lb in range(NB):
                    u_tiles = []
                    srel_slab = astage.tile([P, 4, D], f32, tag="srel", bufs=2,
                                            name="srel_slab")
                    nc.sync.dma_start(srel_slab[:], s_sp[lb])
                    rs_slab = astage.tile([P, 4, D], f32, tag="rs", bufs=2,
                                          name="rs_slab")
                    for mc in range(NLC):
                        psc = ps_mm.tile([P, 512], f32, tag="mm", name="psc")
                        for k in range(ND):
                            nc.tensor.matmul(psc[:], t_kt[:, k, bass.ts(mc, P)],
                                             t_qt[:, k, bass.ts(lb, 512)],
                                             start=(k == 0), stop=(k == ND - 1))
                        ut = upool.tile([P, 512], f32r, tag="u", name=f"u_{lb}_{mc}")
                        nc.scalar.activation(ut[:], psc[:], AF.Exp, scale=SCALE)
                        u_tiles.append(ut)

                    pden = ps_den.tile([1, 512], f32, tag="den", name="pden")
                    for mc in range(NLC):
                        nc.tensor.matmul(pden[:], t_oc[:], u_tiles[mc][:],
                                         start=(mc == 0), stop=(mc == NLC - 1))
                    den_row = small.tile([1, 512], f32r, tag="denr", name="den_row")
                    nc.scalar.copy(den_row[:], pden[:])

                    prc = ps_rec.tile([P, 4, 2], f32, tag="rec", name="prc")
                    for c in range(4):
                        nc.tensor.matmul(prc[:, c, :], den_row[:, bass.ts(c, P)],
                                         t_o2[:], start=True, stop=True)
                    t_rec = small.tile([P, 4], f32, tag="recs", name="t_rec")
                    nc.vector.reciprocal(t_rec[:], prc[:, :, 0])

                    t_avt = avtp.tile([P, ND, 512], f32r, tag="avt", name="t_avt")
                    for dc in range(ND):
                        pav = ps_mm.tile([P, 512], f32, tag="mm", name="pav")
                        for mc in range(NLC):
                            nc.tensor.matmul(pav[:], t_v[:, mc, bass.ts(dc, P)],
                                             u_tiles[mc][:],
                                             start=(mc == 0), stop=(mc == NLC - 1))
                        nc.scalar.copy(t_avt[:, dc, :], pav[:])

                    for c in range(4):
                        lc = lb * 4 + c
                        pwo = ps_mm.tile([P, D], f32, tag="mm", name="pwo")
                        for k in range(ND):
                            nc.tensor.matmul(pwo[:], t_avt[:, k, bass.ts(c, P)],
                                             t_wo[:, k, :],
                                             start=(k == 0), stop=False)
                        nc.tensor.matmul(pwo[:], den_row[:, bass.ts(c, P)],
                                         t_row["bo_row"][:], start=False, stop=True)
                        nc.vector.scalar_tensor_tensor(
                            rs_slab[:, c, :], pwo[:], t_rec[:, c:c + 1],
                            srel_slab[:, c, :],
                            op0=ALU.mult, op1=ALU.add,
                            accum_out=t_sum1a[:, lc:lc + 1])
                        t_scr = astage.tile([P, D], f32, tag="sqscr", name="t_scr")
                        nc.vector.scalar_tensor_tensor(
                            t_scr[:], rs_slab[:, c, :], 1.0, rs_slab[:, c, :],
                            op0=ALU.mult, op1=ALU.mult,
                            accum_out=t_ssq1a[:, lc:lc + 1])
                    nc.gpsimd.dma_start(h_sp[lb], rs_slab[:])
            es_qkv.close()  # QT/KT/V freed

            # ---- phase 5: FFN + LN2 + trend ----
            with tc.tile_pool(name="ffnw", bufs=1) as ffnw, \
                 tc.tile_pool(name="ffn", bufs=1) as ffn, \
                 tc.tile_pool(name="fstage", bufs=2) as fstage, \
                 tc.tile_pool(name="ps_t2", bufs=2, space="PSUM") as ps_t2:
                t_w1 = ffnw.tile([P, ND, D], f32r, name="t_w1f")
                t_w2 = ffnw.tile([P, ND, D], f32r, name="t_w2f")
                nc.sync.dma_start(
                    t_w1[:], ws["w1"].rearrange("(k p) n -> p k n", p=P).bitcast(f32r))
                nc.sync.dma_start(
                    t_w2[:], ws["w2"].rearrange("(k p) n -> p k n", p=P).bitcast(f32r))
                t_h = ffnw.tile([P, NLC, D], f32, name="t_hf")
                for lb in range(NB):
                  rrel_slab = fstage.tile([P, 4, D], f32, tag="rrel", bufs=3,
                                          name="rrel_slab")
                  nc.sync.dma_start(rrel_slab[:], h_sp[lb])
                  t_istd4, t_nmi4, t_negm4 = _ln_block(
                      nc, small, t_sum1a[:, lb * 4:lb * 4 + 4],
                      t_ssq1a[:, lb * 4:lb * 4 + 4], t_eps)
                  for c in range(4):
                    lc = lb * 4 + c
                    nc.vector.tensor_scalar(t_h[:, lc, :], rrel_slab[:, c, :],
                                            t_negm4[:, c:c + 1], t_istd4[:, c:c + 1],
                                            op0=ALU.add, op1=ALU.mult)
                    if apply_g1:
                        nc.vector.tensor_tensor(t_h[:, lc, :], t_h[:, lc, :],
                                                t_gb["g1b"][:], ALU.mult)
                        nc.vector.tensor_tensor(t_h[:, lc, :], t_h[:, lc, :],
                                                t_gb["be1b"][:], ALU.add)

                t_ht = ffn.tile([P, ND, L], f32r, name="t_htf")
                for lc in range(NLC):
                    for dc in range(ND):
                        pht = ps_t2.tile([P, P], f32, tag="pt2", name="pht")
                        nc.tensor.matmul(pht[:], t_h[:, lc, bass.ts(dc, P)],
                                         t_id.bitcast(f32), start=True, stop=True)
                        nc.scalar.copy(t_ht[:, dc, bass.ts(lc, P)], pht[:])

                t_rt = ffn.tile([P, ND, L], f32r, name="t_rtf")
                for lb in range(NB):
                    # ff1 for this l-block
                    for dc in range(ND):
                        pf = ps_mm.tile([P, 512], f32, tag="mm", name="pf")
                        for k in range(ND):
                            nc.tensor.matmul(pf[:], t_w1[:, k, bass.ts(dc, P)],
                                             t_ht[:, k, bass.ts(lb, 512)],
                                             start=(k == 0), stop=(k == ND - 1))
                        nc.scalar.activation(t_rt[:, dc, bass.ts(lb, 512)], pf[:],
                                             AF.Relu, bias=t_b["b1"][:, dc:dc + 1])
                    trel_slab = fstage.tile([P, 4, D], f32, tag="trel", bufs=2,
                                            name="trel_slab")
                    nc.sync.dma_start(trel_slab[:], trend_sp[lb])
                    t_sum2b = small.tile([P, 4], f32, tag="sum2b", name="t_sum2b")
                    t_ssq2b = small.tile([P, 4], f32, tag="ssq2b", name="t_ssq2b")
                    res_list = []
                    for c in range(4):
                        lc = lb * 4 + c
                        pf2 = ps_mm.tile([P, D], f32, tag="mm", name="pf2")
                        for k in range(ND):
                            nc.tensor.matmul(pf2[:], t_rt[:, k, bass.ts(lc, P)],
                                             t_w2[:, k, :],
                                             start=(k == 0), stop=False)
                        nc.tensor.matmul(pf2[:], t_or[:], t_row["bb2_row"][:],
                                         start=False, stop=True)
                        t_res = fstage.tile([P, D], f32, tag="res2", bufs=5,
                                            name="t_res2")
                        nc.vector.scalar_tensor_tensor(
                            t_res[:], pf2[:], 1.0, t_h[:, lc, :],
                            op0=ALU.mult, op1=ALU.add,
                            accum_out=t_sum2b[:, c:c + 1])
                        t_scr = fstage.tile([P, D], f32, tag="sqscr2", name="t_scr2")
                        nc.scalar.activation(t_scr[:], t_res[:], AF.Square,
                                             accum_out=t_ssq2b[:, c:c + 1])
                        res_list.append(t_res)
                    t_istd4, t_nmi4, t_negm4 = _ln_block(
                        nc, small, t_sum2b, t_ssq2b, t_eps)
                    for c in range(4):
                        lc = lb * 4 + c
                        t_h2 = fstage.tile([P, D], f32, tag="h2out", name="t_h2")
                        if c % 2 == 1:
                            nc.scalar.activation(t_h2[:], res_list[c][:], AF.Identity,
                                                 scale=t_istd4[:, c:c + 1],
                                                 bias=t_nmi4[:, c:c + 1])
                        else:
                            nc.vector.tensor_scalar(t_h2[:], res_list[c][:],
                                                    t_negm4[:, c:c + 1],
                                                    t_istd4[:, c:c + 1],
                                                    op0=ALU.add, op1=ALU.mult)
                        if apply_g2:
                            nc.vector.tensor_tensor(t_h2[:], t_h2[:],
                                                    t_gb["g2b"][:], ALU.mult)
                            nc.vector.tensor_tensor(t_h2[:], t_h2[:],
                                                    t_gb["be2b"][:], ALU.add)
                        t_out = fstage.tile([P, D], f32, tag="outst", name="t_out")
                        eng = nc.gpsimd if c % 2 == 0 else nc.vector
                        eng.tensor_tensor(t_out[:], t_h2[:], trel_slab[:, c, :],
                                          ALU.add)
                        nc.sync.dma_start(out_c[lc], t_out[:])
        finally:
            es_qkv.close()

    nc.compile()
    return nc


def _consts(inp):
    bdiag, bup, bdown = _band_blocks()
    cpack = np.zeros((P, 526), np.float32)
    cpack[:, 0:128] = bdiag
    cpack[:, 128:256] = bup
    cpack[:, 256:384] = bdown
    cpack[:, 384:512] = np.eye(P, dtype=np.float32)
    cpack[:, 512] = 1.0                      # ones_col
    cpack[:, 513] = EPS
    cpack[:, 514:518] = inp["bq"].reshape(ND, P).T
    cpack[:, 518:522] = inp["bk"].reshape(ND, P).T
    cpack[:, 522:526] = inp["bb1"].reshape(ND, P).T
    rpack = np.zeros((1, 1666), np.float32)
    rpack[0, 0:512] = inp["bv"]
    rpack[0, 512:1024] = inp["bo"]
    rpack[0, 1024:1536] = inp["bb2"]
    rpack[0, 1536:1664] = 1.0               # ones_row
    rpack[0, 1664:1666] = 1.0               # ones2
    return {
        "wq": inp["wq"], "wk": inp["wk"], "wv": inp["wv"],
        "wo": inp["wo"], "w1": inp["w1"], "w2": inp["w2"],
        "cpack": cpack, "rpack": rpack,
    }


def _prepare(inputs):
    inp = {k: np.ascontiguousarray(np.asarray(v, dtype=np.float32))
           for k, v in inputs.items()}
    x = inp["x"]                      # [8, 2048, 512]
    assert x.shape == (B_, L, D)

    apply_g1 = not (np.allclose(inp["g1"], 1.0) and np.allclose(inp["be1"], 0.0))
    apply_g2 = not (np.allclose(inp["g2"], 1.0) and np.allclose(inp["be2"], 0.0))

    key = (apply_g1, apply_g2)
    if key not in _CACHE:
        _CACHE[key] = _build(apply_g1, apply_g2)
    nc = _CACHE[key]

    consts = _consts(inp)
    if apply_g1:
        consts["g1b"] = np.tile(inp["g1"].reshape(1, D), (P, 1))
        consts["be1b"] = np.tile(inp["be1"].reshape(1, D), (P, 1))
    if apply_g2:
        consts["g2b"] = np.tile(inp["g2"].reshape(1, D), (P, 1))
        consts["be2b"] = np.tile(inp["be2"].reshape(1, D), (P, 1))
    consts = {k: np.ascontiguousarray(v, dtype=np.float32) for k, v in consts.items()}
    in_maps = [dict(consts, x=np.ascontiguousarray(x[i])) for i in range(B_)]
    return nc, in_maps


def kernel(**inputs):
    nc, in_maps = _prepare(inputs)
    res = run_bass_kernel_spmd(nc, in_maps, core_ids=list(range(B_)))
    return np.stack([res.results[i]["out"] for i in range(B_)], axis=0)


# revision 28
# speedup vs baseline: 1.1612x; 1.0081x over previous
"""Autoformer-style EncoderLayer (series-decomp + single-head attention + FFN)
for Trainium2, data-parallel over batch across 8 NeuronCores.

Per core: one [L=2048, D=512] sequence.
  trend = AvgPool1d(x, k=25, pad=12, count_include_pad=True)
  s     = x - trend                        (banded matmul: S = B @ x, B = I - A)
  Q,K,V = s@wq+bq, s@wk+bk, s@wv+bv
  attn  = softmax(Q K^T / sqrt(D))         (computed transposed: scores^T[m,l])
  h     = LN1(s + attn@V@wo + bo)
  out   = LN2(h + relu(h@w1+bb1)@w2+bb2) + trend

All matmuls run in float32r (fp32 data at bf16 PE rate). Activations flow
"transposed" [d, l] for chained projections; scores are computed transposed so
softmax denominators come from ones-matmuls; attn@V yields AVT [d, l] whose Wo
projection lands back in natural [l, d] layout for the free-dim LayerNorms.
Natural<->transposed layout switches are f32r matmuls against an identity.
trend / seasonal / h are spilled to DRAM to keep SBUF under budget.
"""
import math
import numpy as np
from contextlib import ExitStack

import concourse.bass as bass
import concourse.mybir as mybir
import concourse.tile as tile
from concourse import bacc
from concourse.bass_utils import run_bass_kernel_spmd

P = 128
B_, L, D = 8, 2048, 512
KPOOL, PAD = 25, 12
EPS = 1e-5
SCALE = 1.0 / math.sqrt(D)
NLC = L // P          # 16 l-chunks of 128
NB = L // 512         # 4  l-blocks of 512
ND = D // P           # 4  d-chunks of 128

f32 = mybir.dt.float32
f32r = mybir.dt.float32r
AF = mybir.ActivationFunctionType
ALU = mybir.AluOpType

_CACHE = {}


def _band_blocks():
    i = np.arange(P)[:, None]
    j = np.arange(P)[None, :]
    a = (np.abs(i - j) <= PAD).astype(np.float32) / KPOOL
    bdiag = np.eye(P, dtype=np.float32) - a
    bup = -((i - j) >= (P - PAD)).astype(np.float32) / KPOOL   # rows chunk c-1, cols chunk c
    bdown = bup.T.copy()                                       # rows chunk c+1, cols chunk c
    return bdiag, bup, bdown


def _ln_block(nc, small, t_sum, t_ssq, t_eps):
    """Per-block LayerNorm stats on [P, 4]: returns (istd, nmi, negmean)."""
    t_mean = small.tile([P, 4], f32, tag="lbm", name="tb_mean")
    nc.vector.tensor_scalar_mul(t_mean[:], t_sum[:], 1.0 / D)
    t_m2 = small.tile([P, 4], f32, tag="lbm2", name="tb_m2")
    nc.vector.tensor_tensor(t_m2[:], t_mean[:], t_mean[:], ALU.mult)
    t_var = small.tile([P, 4], f32, tag="lbv", name="tb_var")
    nc.vector.scalar_tensor_tensor(t_var[:], t_ssq[:], 1.0 / D, t_m2[:],
                                   op0=ALU.mult, op1=ALU.subtract)
    t_sd = small.tile([P, 4], f32, tag="lbsd", name="tb_sd")
    nc.scalar.activation(t_sd[:], t_var[:], AF.Sqrt, bias=t_eps[:])
    t_istd = small.tile([P, 4], f32, tag="lbi", name="tb_istd")
    nc.vector.reciprocal(t_istd[:], t_sd[:])
    t_nmi = small.tile([P, 4], f32, tag="lbn", name="tb_nmi")
    nc.vector.scalar_tensor_tensor(t_nmi[:], t_mean[:], -1.0, t_istd[:],
                                   op0=ALU.mult, op1=ALU.mult)
    t_negm = small.tile([P, 4], f32, tag="lbng", name="tb_negm")
    nc.vector.tensor_scalar_mul(t_negm[:], t_mean[:], -1.0)
    return t_istd, t_nmi, t_negm


def _ln_chunk(nc, small, t_sum, t_ssq, t_eps):
    """Per-chunk LayerNorm stats on [P, 1]: returns (istd, -mean*istd)."""
    t_mean = small.tile([P, 1], f32, tag="lnm", name="t_mean")
    nc.vector.tensor_scalar_mul(t_mean[:], t_sum[:], 1.0 / D)
    t_var = small.tile([P, 1], f32, tag="lnv", name="t_var")
    # var = ssq/D - mean^2  ->  (ssq/D) + (-mean*mean) via two ops
    t_m2 = small.tile([P, 1], f32, tag="lnm2", name="t_m2")
    nc.vector.tensor_tensor(t_m2[:], t_mean[:], t_mean[:], ALU.mult)
    nc.vector.scalar_tensor_tensor(t_var[:], t_ssq[:], 1.0 / D, t_m2[:],
                                   op0=ALU.mult, op1=ALU.subtract)
    t_sd = small.tile([P, 1], f32, tag="lnsd", name="t_sd")
    nc.scalar.activation(t_sd[:], t_var[:], AF.Sqrt, bias=t_eps[:])
    t_istd = small.tile([P, 1], f32, tag="lni", name="t_istd")
    nc.vector.reciprocal(t_istd[:], t_sd[:])
    t_nmi = small.tile([P, 1], f32, tag="lnn", name="t_nmi")
    nc.vector.scalar_tensor_tensor(t_nmi[:], t_mean[:], -1.0, t_istd[:],
                                   op0=ALU.mult, op1=ALU.mult)
    t_negm = small.tile([P, 1], f32, tag="lnng", name="t_negm")
    nc.vector.tensor_scalar_mul(t_negm[:], t_mean[:], -1.0)
    return t_istd, t_nmi, t_negm


def _build(apply_g1, apply_g2):
    nc = bacc.Bacc("TRN2", target_bir_lowering=False, debug=False)

    def din(name, shape):
        return nc.dram_tensor(name, list(shape), f32, kind="ExternalInput").ap()

    x = din("x", (L, D))
    ws = {n: din(n, (D, D)) for n in ["wq", "wk", "wv", "wo", "w1", "w2"]}
    cpack = din("cpack", (P, 526))
    rpack = din("rpack", (1, 1666))
    gb = {}
    if apply_g1:
        gb["g1b"] = din("g1b", (P, D))
        gb["be1b"] = din("be1b", (P, D))
    if apply_g2:
        gb["g2b"] = din("g2b", (P, D))
        gb["be2b"] = din("be2b", (P, D))

    out = nc.dram_tensor("out", [L, D], f32, kind="ExternalOutput").ap()
    out_c = out.rearrange("(l p) d -> l p d", p=P)

    with tile.TileContext(nc) as tc, ExitStack() as ctx:
        misc = ctx.enter_context(tc.tile_pool(name="misc", bufs=1))
        small = ctx.enter_context(tc.tile_pool(name="small", bufs=4))
        dram = ctx.enter_context(tc.tile_pool(name="dram", bufs=1, space="DRAM"))
        ps_mm = ctx.enter_context(tc.tile_pool(name="ps_mm", bufs=5, space="PSUM"))

        # ---- constants (two packed DMAs) ----
        t_cp = misc.tile([P, 526], f32r, name="t_cp")
        nc.sync.dma_start(t_cp[:], cpack.bitcast(f32r))
        t_rp = misc.tile([1, 1666], f32r, name="t_rp")
        nc.sync.dma_start(t_rp[:], rpack.bitcast(f32r))
        t_bd = t_cp[:, 0:128]
        t_bu = t_cp[:, 128:256]
        t_bn = t_cp[:, 256:384]
        t_id = t_cp[:, 384:512]
        t_oc = t_cp[:, 512:513]
        t_eps = t_cp[:, 513:514].bitcast(f32)
        t_row = {"bv_row": t_rp[:, 0:512], "bo_row": t_rp[:, 512:1024],
                 "bb2_row": t_rp[:, 1024:1536]}
        t_or = t_rp[:, 1536:1664]
        t_o2 = t_rp[:, 1664:1666]
        t_b = {"bq": t_cp[:, 514:518].bitcast(f32),
               "bk": t_cp[:, 518:522].bitcast(f32),
               "b1": t_cp[:, 522:526].bitcast(f32)}
        t_sum1a = misc.tile([P, NLC], f32, name="t_sum1a")
        t_ssq1a = misc.tile([P, NLC], f32, name="t_ssq1a")
        t_gb = {}
        for n in gb:
            t_gb[n] = misc.tile([P, D], f32, name=f"t_{n}")
            nc.sync.dma_start(t_gb[n][:], gb[n][:])

        trend_sp = dram.tile([NB, P, 4, D], f32)
        s_sp = dram.tile([NB, P, 4, D], f32)
        h_sp = dram.tile([NB, P, 4, D], f32)

        es_qkv = ExitStack()
        try:
            qkv = es_qkv.enter_context(tc.tile_pool(name="qkv", bufs=1))
            t_qt = qkv.tile([P, ND, L], f32r, name="t_qt")
            t_kt = qkv.tile([P, ND, L], f32r, name="t_kt")
            t_v = qkv.tile([P, NLC, D], f32r, name="t_v")

            # ---- phases 1-3 fused, streamed per l-block:
            # banded seasonal -> trend -> S^T block -> QT/KT/V block
            x_cview = x.rearrange("(l p) d -> p l d", p=P)
            with tc.tile_pool(name="wqkv", bufs=1) as wqkv, \
                 tc.tile_pool(name="xwin", bufs=6) as xwin, \
                 tc.tile_pool(name="sslab", bufs=2) as sslab, \
                 tc.tile_pool(name="trslab", bufs=2) as trslab, \
                 tc.tile_pool(name="stbp", bufs=2) as stbp, \
                 tc.tile_pool(name="ps_t", bufs=2, space="PSUM") as ps_t:
                x_ch = {}

                def get_x(j):
                    if j not in x_ch:
                        t = xwin.tile([P, D], f32r, tag="xw", name=f"xw{j}")
                        nc.sync.dma_start(t[:], x_cview[:, j, :].bitcast(f32r))
                        x_ch[j] = t
                    return x_ch[j]

                for j in range(5):      # prefetch ahead of the weight DMAs
                    get_x(j)
                t_w = {}
                for n in ["wq", "wk", "wv"]:
                    t_w[n] = wqkv.tile([P, ND, D], f32r, name=f"t_w_{n}")
                    nc.sync.dma_start(
                        t_w[n][:], ws[n].rearrange("(k p) n -> p k n", p=P).bitcast(f32r))

                for lb in range(NB):
                    # (a) banded S + trend into per-block slabs
                    s_slab = sslab.tile([P, 4, D], f32, tag="ss", name="s_slab")
                    tr_slab = trslab.tile([P, 4, D], f32, tag="tr", name="tr_slab")
                    for c in range(4):
                        lc = lb * 4 + c
                        pss = ps_mm.tile([P, D], f32, tag="mm", name="pss")
                        nbrs = [(lc - 1, t_bu), (lc, t_bd), (lc + 1, t_bn)]
                        nbrs = [(j, t) for j, t in nbrs if 0 <= j < NLC]
                        for i, (j, tb) in enumerate(nbrs):
                            nc.tensor.matmul(pss[:], tb[:], get_x(j)[:],
                                             start=(i == 0), stop=(i == len(nbrs) - 1))
                        nc.vector.tensor_copy(s_slab[:, c, :], pss[:])
                        nc.vector.tensor_tensor(tr_slab[:, c, :],
                                                get_x(lc)[:].bitcast(f32),
                                                s_slab[:, c, :],
                                                ALU.subtract)
                    nc.gpsimd.dma_start(s_sp[lb], s_slab[:])
                    nc.gpsimd.dma_start(trend_sp[lb], tr_slab[:])

                    # (b) S^T block [d, l-block] via identity matmuls
                    stb = stbp.tile([P, ND, 512], f32r, tag="stb", name="stb")
                    for c in range(4):
                        for dc in range(ND):
                            pst = ps_t.tile([P, P], f32, tag="pt", name="pst")
                            nc.tensor.matmul(pst[:], s_slab[:, c, bass.ts(dc, P)],
                                             t_id.bitcast(f32), start=True, stop=True)
                            nc.scalar.copy(stb[:, dc, bass.ts(c, P)], pst[:])

                    # (c) QT/KT for this l-block
                    for tdst, wname, bname in [(t_qt, "wq", "bq"), (t_kt, "wk", "bk")]:
                        for dc in range(ND):
                            pq = ps_mm.tile([P, 512], f32, tag="mm", name="pq")
                            for k in range(ND):
                                nc.tensor.matmul(pq[:], t_w[wname][:, k, bass.ts(dc, P)],
                                                 stb[:, k, :],
                                                 start=(k == 0), stop=(k == ND - 1))
                            nc.scalar.activation(tdst[:, dc, bass.ts(lb, 512)], pq[:],
                                                 AF.Identity, bias=t_b[bname][:, dc:dc + 1])
                    # (d) V for this block's 4 m-chunks
                    for c in range(4):
                        mc = lb * 4 + c
                        pv = ps_mm.tile([P, D], f32, tag="mm", name="pv")
                        for k in range(ND):
                            nc.tensor.matmul(pv[:], stb[:, k, bass.ts(c, P)],
                                             t_w["wv"][:, k, :],
                                             start=(k == 0), stop=False)
                        nc.tensor.matmul(pv[:], t_or[:], t_row["bv_row"][:],
                                         start=False, stop=True)
                        nc.scalar.copy(t_v[:, mc, :], pv[:])

            # ---- phase 4: attention + LN1 ----
            with tc.tile_pool(name="upool", bufs=18) as upool, \
                 tc.tile_pool(name="wo_pool", bufs=1) as wo_pool, \
                 tc.tile_pool(name="astage", bufs=2) as astage, \
                 tc.tile_pool(name="avtp", bufs=1) as avtp, \
                 tc.tile_pool(name="ps_den", bufs=2, space="PSUM") as ps_den, \
                 tc.tile_pool(name="ps_rec", bufs=1, space="PSUM") as ps_rec:
                t_wo = wo_pool.tile([P, ND, D], f32r, name="t_wo")
                nc.sync.dma_start(
                    t_wo[:], ws["wo"].rearrange("(k p) n -> p k n", p=P).bitcast(f32r))

                for lb in range(NB):
                    u_tiles = []
                    srel_slab = astage.tile([P, 4, D], f32, tag="srel", bufs=2,
                                            name="srel_slab")
                    nc.sync.dma_start(srel_slab[:], s_sp[lb])
                    rs_slab = astage.tile([P, 4, D], f32, tag="rs", bufs=2,
                                          name="rs_slab")
                    for mc in range(NLC):
                        psc = ps_mm.tile([P, 512], f32, tag="mm", name="psc")
                        for k in range(ND):
                            nc.tensor.matmul(psc[:], t_kt[:, k, bass.ts(mc, P)],
                                             t_qt[:, k, bass.ts(lb, 512)],
                                             start=(k == 0), stop=(k == ND - 1))
                        ut = upool.tile([P, 512], f32r, tag="u", name=f"u_{lb}_{mc}")
                        nc.scalar.activation(ut[:], psc[:], AF.Exp, scale=SCALE)
                        u_tiles.append(ut)

                    pden = ps_den.tile([1, 512], f32, tag="den", name="pden")
                    for mc in range(NLC):
                        nc.tensor.matmul(pden[:], t_oc[:], u_tiles[mc][:],
                                         start=(mc == 0), stop=(mc == NLC - 1))
                    den_row = small.tile([1, 512], f32r, tag="denr", name="den_row")
                    nc.scalar.copy(den_row[:], pden[:])

                    prc = ps_rec.tile([P, 4, 2], f32, tag="rec", name="prc")
                    for c in range(4):
                        nc.tensor.matmul(prc[:, c, :], den_row[:, bass.ts(c, P)],
                                         t_o2[:], start=True, stop=True)
                    t_rec = small.tile([P, 4], f32, tag="recs", name="t_rec")
                    nc.vector.reciprocal(t_rec[:], prc[:, :, 0])

                    t_avt = avtp.tile([P, ND, 512], f32r, tag="avt", name="t_avt")
                    for dc in range(ND):
                        pav = ps_mm.tile([P, 512], f32, tag="mm", name="pav")
                        for mc in range(NLC):
                            nc.tensor.matmul(pav[:], t_v[:, mc, bass.ts(dc, P)],
                                             u_tiles[mc][:],
                                             start=(mc == 0), stop=(mc == NLC - 1))
                        nc.scalar.copy(t_avt[:, dc, :], pav[:])

                    for c in range(4):
                        lc = lb * 4 + c
                        pwo = ps_mm.tile([P, D], f32, tag="mm", name="pwo")
                        for k in range(ND):
                            nc.tensor.matmul(pwo[:], t_avt[:, k, bass.ts(c, P)],
                                             t_wo[:, k, :],
                                             start=(k == 0), stop=False)
                        nc.tensor.matmul(pwo[:], den_row[:, bass.ts(c, P)],
                                         t_row["bo_row"][:], start=False, stop=True)
                        nc.vector.scalar_tensor_tensor(
                            rs_slab[:, c, :], pwo[:], t_rec[:, c:c + 1],
                            srel_slab[:, c, :],
                            op0=ALU.mult, op1=ALU.add,
                            accum_out=t_sum1a[:, lc:lc + 1])
                        t_scr = astage.tile([P, D], f32, tag="sqscr", name="t_scr")
                        nc.vector.scalar_tensor_tensor(
                            t_scr[:], rs_slab[:, c, :], 1.0, rs_slab[:, c, :],
                            op0=ALU.mult, op1=ALU.mult,
                            accum_out=t_ssq1a[:, lc:lc + 1])
                    nc.gpsimd.dma_start(h_sp[lb], rs_slab[:])
            es_qkv.close()  # QT/KT/V freed

            # ---- phase 5: FFN + LN2 + trend ----
            with tc.tile_pool(name="ffnw", bufs=1) as ffnw, \
                 tc.tile_pool(name="ffn", bufs=1) as ffn, \
                 tc.tile_pool(name="fstage", bufs=2) as fstage, \
                 tc.tile_pool(name="ps_t2", bufs=2, space="PSUM") as ps_t2:
                t_w1 = ffnw.tile([P, ND, D], f32r, name="t_w1f")
                t_w2 = ffnw.tile([P, ND, D], f32r, name="t_w2f")
                nc.sync.dma_start(
                    t_w1[:], ws["w1"].rearrange("(k p) n -> p k n", p=P).bitcast(f32r))
                nc.sync.dma_start(
                    t_w2[:], ws["w2"].rearrange("(k p) n -> p k n", p=P).bitcast(f32r))
                t_h = ffnw.tile([P, NLC, D], f32, name="t_hf")
                for lb in range(NB):
                  rrel_slab = fstage.tile([P, 4, D], f32, tag="rrel", bufs=3,
                                          name="rrel_slab")
                  nc.sync.dma_start(rrel_slab[:], h_sp[lb])
                  t_istd4, t_nmi4, t_negm4 = _ln_block(
                      nc, small, t_sum1a[:, lb * 4:lb * 4 + 4],
                      t_ssq1a[:, lb * 4:lb * 4 + 4], t_eps)
                  for c in range(4):
                    lc = lb * 4 + c
                    nc.vector.tensor_scalar(t_h[:, lc, :], rrel_slab[:, c, :],
                                            t_negm4[:, c:c + 1], t_istd4[:, c:c + 1],
                                            op0=ALU.add, op1=ALU.mult)
                    if apply_g1:
                        nc.vector.tensor_tensor(t_h[:, lc, :], t_h[:, lc, :],
                                                t_gb["g1b"][:], ALU.mult)
                        nc.vector.tensor_tensor(t_h[:, lc, :], t_h[:, lc, :],
                                                t_gb["be1b"][:], ALU.add)

                t_ht = ffn.tile([P, ND, L], f32r, name="t_htf")
                for lc in range(NLC):
                    for dc in range(ND):
                        pht = ps_t2.tile([P, P], f32, tag="pt2", name="pht")
                        nc.tensor.matmul(pht[:], t_h[:, lc, bass.ts(dc, P)],
                                         t_id.bitcast(f32), start=True, stop=True)
                        nc.scalar.copy(t_ht[:, dc, bass.ts(lc, P)], pht[:])

                t_rt = ffn.tile([P, ND, L], f32r, name="t_rtf")
                for lb in range(NB):
                    # ff1 for this l-block
                    for dc in range(ND):
                        pf = ps_mm.tile([P, 512], f32, tag="mm", name="pf")
                        for k in range(ND):
                            nc.tensor.matmul(pf[:], t_w1[:, k, bass.ts(dc, P)],
                                             t_ht[:, k, bass.ts(lb, 512)],
                                             start=(k == 0), stop=(k == ND - 1))
                        nc.scalar.activation(t_rt[:, dc, bass.ts(lb, 512)], pf[:],
                                             AF.Relu, bias=t_b["b1"][:, dc:dc + 1])
                    trel_slab = fstage.tile([P, 4, D], f32, tag="trel", bufs=2,
                                            name="trel_slab")
                    nc.sync.dma_start(trel_slab[:], trend_sp[lb])
                    t_sum2b = small.tile([P, 4], f32, tag="sum2b", name="t_sum2b")
                    t_ssq2b = small.tile([P, 4], f32, tag="ssq2b", name="t_ssq2b")
                    res_list = []
                    for c in range(4):
                        lc = lb * 4 + c
                        pf2 = ps_mm.tile([P, D], f32, tag="mm", name="pf2")
                        for k in range(ND):
                            nc.tensor.matmul(pf2[:], t_rt[:, k, bass.ts(lc, P)],
                                             t_w2[:, k, :],
                                             start=(k == 0), stop=False)
                        nc.tensor.matmul(pf2[:], t_or[:], t_row["bb2_row"][:],
                                         start=False, stop=True)
                        t_res = fstage.tile([P, D], f32, tag="res2", bufs=5,
                                            name="t_res2")
                        nc.vector.scalar_tensor_tensor(
                            t_res[:], pf2[:], 1.0, t_h[:, lc, :],
                            op0=ALU.mult, op1=ALU.add,
                            accum_out=t_sum2b[:, c:c + 1])
                        t_scr = fstage.tile([P, D], f32, tag="sqscr2", name="t_scr2")
                        nc.scalar.activation(t_scr[:], t_res[:], AF.Square,
                                             accum_out=t_ssq2b[:, c:c + 1])
                        res_list.append(t_res)
                    t_istd4, t_nmi4, t_negm4 = _ln_block(
                        nc, small, t_sum2b, t_ssq2b, t_eps)
                    for c in range(4):
                        lc = lb * 4 + c
                        t_h2 = fstage.tile([P, D], f32, tag="h2out", name="t_h2")
                        if c % 2 == 1:
                            nc.scalar.activation(t_h2[:], res_list[c][:], AF.Identity,
                                                 scale=t_istd4[:, c:c + 1],
                                                 bias=t_nmi4[:, c:c + 1])
                        else:
                            nc.vector.tensor_scalar(t_h2[:], res_list[c][:],
                                                    t_negm4[:, c:c + 1],
                                                    t_istd4[:, c:c + 1],
                                                    op0=ALU.add, op1=ALU.mult)
                        if apply_g2:
                            nc.vector.tensor_tensor(t_h2[:], t_h2[:],
                                                    t_gb["g2b"][:], ALU.mult)
                            nc.vector.tensor_tensor(t_h2[:], t_h2[:],
                                                    t_gb["be2b"][:], ALU.add)
                        t_out = fstage.tile([P, D], f32, tag="outst", name="t_out")
                        eng = nc.gpsimd if c % 2 == 0 else nc.vector
                        eng.tensor_tensor(t_out[:], t_h2[:], trel_slab[:, c, :],
                                          ALU.add)
                        nc.sync.dma_start(out_c[lc], t_out[:])
        finally:
            es_qkv.close()

    nc.compile()
    return nc


def _consts(inp):
    bdiag, bup, bdown = _band_blocks()
    cpack = np.zeros((P, 526), np.float32)
    cpack[:, 0:128] = bdiag
    cpack[:, 128:256] = bup
    cpack[:, 256:384] = bdown
    cpack[:, 384:512] = np.eye(P, dtype=np.float32)
    cpack[:, 512] = 1.0                      # ones_col
    cpack[:, 513] = EPS
    cpack[:, 514:518] = inp["bq"].reshape(ND, P).T
    cpack[:, 518:522] = inp["bk"].reshape(ND, P).T
    cpack[:, 522:526] = inp["bb1"].reshape(ND, P).T
    rpack = np.zeros((1, 1666), np.float32)
    rpack[0, 0:512] = inp["bv"]
    rpack[0, 512:1024] = inp["bo"]
    rpack[0, 1024:1536] = inp["bb2"]
    rpack[0, 1536:1664] = 1.0               # ones_row
    rpack[0, 1664:1666] = 1.0               # ones2
    return {
        "wq": inp["wq"], "wk": inp["wk"], "wv": inp["wv"],
        "wo": inp["wo"], "w1": inp["w1"], "w2": inp["w2"],
        "cpack": cpack, "rpack": rpack,
    }


def _prepare(inputs):
    inp = {k: np.ascontiguousarray(np.asarray(v, dtype=np.float32))
           for k, v in inputs.items()}
    x = inp["x"]                      # [8, 2048, 512]
    assert x.shape == (B_, L, D)

    apply_g1 = not (np.allclose(inp["g1"], 1.0) and np.allclose(inp["be1"], 0.0))
    apply_g2 = not (np.allclose(inp["g2"], 1.0) and np.allclose(inp["be2"], 0.0))

    key = (apply_g1, apply_g2)
    if key not in _CACHE:
        _CACHE[key] = _build(apply_g1, apply_g2)
    nc = _CACHE[key]

    consts = _consts(inp)
    if apply_g1:
        consts["g1b"] = np.tile(inp["g1"].reshape(1, D), (P, 1))
        consts["be1b"] = np.tile(inp["be1"].reshape(1, D), (P, 1))
    if apply_g2:
        consts["g2b"] = np.tile(inp["g2"].reshape(1, D), (P, 1))
        consts["be2b"] = np.tile(inp["be2"].reshape(1, D), (P, 1))
    consts = {k: np.ascontiguousarray(v, dtype=np.float32) for k, v in consts.items()}
    in_maps = [dict(consts, x=np.ascontiguousarray(x[i])) for i in range(B_)]
    return nc, in_maps


def kernel(**inputs):
    nc, in_maps = _prepare(inputs)
    res = run_bass_kernel_spmd(nc, in_maps, core_ids=list(range(B_)))
    return np.stack([res.results[i]["out"] for i in range(B_)], axis=0)
